# revision 30
# baseline (speedup 1.0000x reference)
"""Trainium2 Bass kernel for nn_FreqCrossAttention.

Sharding: 8 cores = 4 batches x 2 head-groups (8 heads each), with the
DFT work split across each batch pair: the even core computes the DFT of
LN(query), the odd core the DFT of key_value (LN is switched off via a
per-core mask input so the program stays uniform). The two spectra are
exchanged with a pairwise chunked AllGather, after which both cores run
the 6 projections for their own head-group, attention, iDFT and the
row-parallel W_o partial. Host sums the pair per batch.

All matmuls bf16; everything SBUF-resident except the Q/K staging and
the spectra exchange, which go through DRAM.
"""
import math
import numpy as np
import ml_dtypes

B, L, E, H = 4, 2048, 1024, 16
D = E // H            # 64
Lf = L // 2 + 1       # 1025
FP = 1026             # padded frequency dim (= 3*342)
NH = 8                # heads per core
P = 128
CH = 342              # DFT/proj frequency chunk
NCH = 3
LT = 16               # L tiles of 128
ET = 8                # e-chunks of E
EPS = 1e-5
SQL = math.sqrt(L)
# m-tiles (keys), chunk-aligned: per chunk (0,128),(128,128),(256,86);
# the final tile is 85 rows (m=940..1024) so the pad key row 1025 is
# never part of the softmax.
MTI = []
for _c in range(NCH):
    last = 85 if _c == NCH - 1 else 86
    MTI += [(_c * CH, 128), (_c * CH + 128, 128), (_c * CH + 256, last)]
# l-tiles (queries/outputs) incl pad col 1025
FTI = [(i * P, P) for i in range(8)] + [(1024, 2)]

_CACHE = {}


def _dft_consts():
    f = np.arange(FP)
    t = np.arange(L)
    ang = 2.0 * np.pi * np.outer(t, f) / L            # [L, FP]
    s = 1.0 / math.sqrt(L)
    FcT = (np.cos(ang) * s).astype(np.float32)        # rhs for rfft [L, FP]
    FsT = (-np.sin(ang) * s).astype(np.float32)
    FcT[:, Lf:] = 0.0
    FsT[:, Lf:] = 0.0
    cw = np.where((f == 0) | (f == L // 2), 1.0, 2.0)[:, None]
    GcT = (cw * np.cos(ang.T) * s).astype(np.float32)  # [FP, L]
    GsT = (-cw * np.sin(ang.T) * s).astype(np.float32)
    GcT[Lf:, :] = 0.0
    GsT[Lf:, :] = 0.0
    return FcT, FsT, GcT, GsT


def _build():
    import concourse.bass as bass
    import concourse.bacc as bacc
    import concourse.mybir as mybir
    import concourse.tile as tile

    F32 = mybir.dt.float32
    BF16 = mybir.dt.bfloat16
    AF = mybir.ActivationFunctionType

    nc = bacc.Bacc("TRN2", debug=False, num_devices=8)

    x_d = nc.dram_tensor("x", [L, E], BF16, kind="ExternalInput")
    lnm_d = nc.dram_tensor("lnm", [P, 1], F32, kind="ExternalInput")
    gamma_d = nc.dram_tensor("gamma", [E, 1], F32, kind="ExternalInput")
    beta_d = nc.dram_tensor("beta", [E, 1], F32, kind="ExternalInput")
    FcT_d = nc.dram_tensor("FcT", [L, FP], BF16, kind="ExternalInput")
    FsT_d = nc.dram_tensor("FsT", [L, FP], BF16, kind="ExternalInput")
    GcT_d = nc.dram_tensor("GcT", [FP, L], BF16, kind="ExternalInput")
    GsT_d = nc.dram_tensor("GsT", [FP, L], BF16, kind="ExternalInput")
    W_d = {}
    for nm in ("qr", "qi", "kr", "ki", "vr", "vi"):
        W_d[nm] = nc.dram_tensor(f"W{nm}", [E, 512], BF16, kind="ExternalInput")
        W_d["b" + nm] = nc.dram_tensor(f"b{nm}", [512, 1], F32, kind="ExternalInput")
    WoT_d = nc.dram_tensor("WoT", [512, E], BF16, kind="ExternalInput")
    out_d = nc.dram_tensor("out", [L, E], F32, kind="ExternalOutput")
    # spectra exchange buffers: rows = ri*1024 + eb*128 + p
    xfo_d = [nc.dram_tensor(f"xfo{c}", [2 * E, CH], BF16) for c in range(NCH)]
    gath_d = [nc.dram_tensor(f"gath{c}", [4 * E, CH], BF16) for c in range(NCH)]
    # Q/K staging (head-interleaved cat layout, rows 0:64 real / 64:128 imag)
    Qcat_d = nc.dram_tensor("Qcat", [P, NH * FP], BF16)
    Kcat_d = nc.dram_tensor("Kcat", [P, NH * FP], BF16)

    with tile.TileContext(nc) as tc:
        with tc.tile_pool(name="persist", bufs=1) as persist:
            Vc = [persist.tile([P, NH * 129], BF16, tag=f"Vc{i}", name=f"Vc{i}")
                  for i in range(len(MTI))]
            # constants (gpsimd DMA queue keeps sync free for x/fct loads)
            eps_t = persist.tile([P, 1], F32)
            nc.vector.memset(eps_t[:], EPS)
            lnm = persist.tile([P, 1], F32)
            nc.gpsimd.dma_start(lnm[:], lnm_d.ap())
            gam8 = persist.tile([P, ET], F32)
            nc.gpsimd.dma_start(gam8[:], gamma_d.ap().rearrange("(c p) one -> p (c one)", p=P))
            bet8 = persist.tile([P, ET], F32)
            nc.gpsimd.dma_start(bet8[:], beta_d.ap().rearrange("(c p) one -> p (c one)", p=P))
            bias_t = {}
            for nm in ("qr", "qi", "kr", "ki"):
                bias_t[nm] = persist.tile([P, 4], F32, tag=f"b{nm}", name=f"b{nm}")
                nc.gpsimd.dma_start(bias_t[nm][:],
                                    W_d["b" + nm].ap().rearrange("(mt p) one -> p (mt one)", p=P))

            def load_w(names, wp):
                Wt = {}
                for nm in names:
                    Wt[nm] = wp.tile([P, ET * 512], BF16, tag=f"W{nm}", name=f"W{nm}")
                    nc.gpsimd.dma_start(
                        Wt[nm][:].rearrange("p (c e) -> p c e", c=ET),
                        W_d[nm].ap().rearrange("(c p) e -> p c e", p=P))
                return Wt

            def load_fslab(fsl, c):
                # split in lc-halves so the first DFT matmuls can start early
                f0 = c * CH
                fct = fsl.tile([P, LT * CH], BF16, tag="fct", name="fct")
                fst = fsl.tile([P, LT * CH], BF16, tag="fst", name="fst")
                hl = LT // 2
                for hf in range(2):
                    cs_ = slice(hf * hl * CH, (hf + 1) * hl * CH)
                    rs_ = slice(hf * hl * P, (hf + 1) * hl * P)
                    nc.sync.dma_start(
                        fct[:, cs_].rearrange("p (lc f) -> p lc f", lc=hl),
                        FcT_d.ap()[rs_, f0:f0 + CH].rearrange("(lc p) f -> p lc f", p=P))
                    nc.sync.dma_start(
                        fst[:, cs_].rearrange("p (lc f) -> p lc f", lc=hl),
                        FsT_d.ap()[rs_, f0:f0 + CH].rearrange("(lc p) f -> p lc f", p=P))
                return fct, fst

            def stage_to_cat(cat_d, stage_r, stage_i, f0):
                # heads h = 2*mt + hh ; real rows 0:64 of cat, imag rows 64:128
                catv = cat_d.ap().rearrange("p (h f) -> p h f", h=NH)
                for hh in range(2):
                    nc.gpsimd.dma_start(
                        catv[0:64, hh::2, f0:f0 + CH],
                        stage_r[hh * 64:(hh + 1) * 64, :].rearrange("p (mt f) -> p mt f", f=CH))
                    nc.gpsimd.dma_start(
                        catv[64:128, hh::2, f0:f0 + CH],
                        stage_i[hh * 64:(hh + 1) * 64, :].rearrange("p (mt f) -> p mt f", f=CH))

            # =================== DFT phase (one path per core) ===================
            with tc.tile_pool(name="qnsp", bufs=1) as qnsp, \
                 tc.tile_pool(name="qin", bufs=2) as qin, \
                 tc.tile_pool(name="lns", bufs=4) as lns, \
                 tc.tile_pool(name="wpool", bufs=1) as wpool, \
                 tc.tile_pool(name="fsl", bufs=1) as fsl, \
                 tc.tile_pool(name="xfp", bufs=1) as xfp, \
                 tc.tile_pool(name="gsl", bufs=2) as gslp, \
                 tc.tile_pool(name="stg", bufs=1) as stg, \
                 tc.tile_pool(name="dps", bufs=2, space="PSUM") as dps, \
                 tc.tile_pool(name="pps", bufs=4, space="PSUM") as pps:
                qns = qnsp.tile([P, LT * E], BF16)

                vbias = {}
                for nm in ("vr", "vi"):
                    vb_row = qin.tile([1, 512], F32, tag="qsl", name="vb_row")
                    nc.gpsimd.dma_start(vb_row[:], W_d["b" + nm].ap().rearrange("e one -> one e"))
                    vb = wpool.tile([P, 512], F32, tag=f"vb{nm}", name=f"vb{nm}")
                    nc.gpsimd.partition_broadcast(vb[:], vb_row[:])
                    vbias[nm] = vb
                Wt = load_w(("kr", "ki", "vr", "vi", "qr", "qi"), wpool)

                cur_slab = load_fslab(fsl, 0)
                nxt_slab = load_fslab(fsl, 1)

                # ---- masked LN (x quarters stream on sync; DVE stats, ACT apply) ----
                for qq in range(8):
                    qsl = qin.tile([P, 2 * E], BF16, tag="qsl", name="qsl")
                    nc.sync.dma_start(
                        qsl[:].rearrange("p (lc e) -> p lc e", lc=2),
                        x_d.ap()[qq * 2 * P:(qq + 1) * 2 * P, :].rearrange(
                            "(lc p) e -> p lc e", p=P))
                    for lq in range(2):
                        lc = qq * 2 + lq
                        qt = qsl[:, lq * E:(lq + 1) * E]
                        st = lns.tile([P, 12], F32, tag="st", name="st")
                        nc.vector.bn_stats(st[:, 0:6], qt[:, 0:512])
                        nc.vector.bn_stats(st[:, 6:12], qt[:, 512:1024])
                        mv = lns.tile([P, 2], F32, tag="mv", name="mv")
                        nc.vector.bn_aggr(mv[:], st[:])
                        sd = lns.tile([P, 1], F32, tag="sd", name="sd")
                        nc.scalar.activation(sd[:], mv[:, 1:2], AF.Sqrt, bias=eps_t[:])
                        istd = lns.tile([P, 1], F32, tag="istd", name="istd")
                        nc.vector.reciprocal(istd[:], sd[:])
                        # LN mask: scale = m*istd + (1-m), bias = -m*mu*istd
                        nc.vector.tensor_scalar_add(istd[:], istd[:], -1.0)
                        nc.vector.tensor_mul(istd[:], istd[:], lnm[:])
                        nc.vector.tensor_scalar_add(istd[:], istd[:], 1.0)
                        nmu = lns.tile([P, 1], F32, tag="nmu", name="nmu")
                        nc.vector.tensor_scalar_mul(nmu[:], mv[:, 0:1], -1.0)
                        nc.vector.tensor_mul(nmu[:], nmu[:], istd[:])
                        nc.vector.tensor_mul(nmu[:], nmu[:], lnm[:])
                        nc.scalar.activation(qns[:, lc * E:(lc + 1) * E], qt, AF.Identity,
                                             bias=nmu[:], scale=istd[:])

                # ---- DFT chunks + pipelined exchange + projections ----
                def proj_chunk(c, xq, gk):
                    """Q/K/V projections for chunk c. xq: [128,(8 eb)*CH] q-spectrum
                    slab (real, imag); gk: kv-spectrum slabs."""
                    xq_r, xq_i = xq
                    xk_r, xk_i = gk
                    qr_st = stg.tile([P, 4 * CH], BF16, tag="qr_st", name="qr_st")
                    qi_st = stg.tile([P, 4 * CH], BF16, tag="qi_st", name="qi_st")
                    kr_st = stg.tile([P, 4 * CH], BF16, tag="kr_st", name="kr_st")
                    ki_st = stg.tile([P, 4 * CH], BF16, tag="ki_st", name="ki_st")
                    for mt in range(4):
                        pqr = pps.tile([P, CH], F32, tag="pp", name="pqr")
                        pqi = pps.tile([P, CH], F32, tag="pp", name="pqi")
                        for ec in range(ET):
                            nc.tensor.matmul(pqr[:], Wt["qr"][:, ec * 512 + mt * P: ec * 512 + (mt + 1) * P],
                                             xq_r[:, ec * CH:(ec + 1) * CH],
                                             start=(ec == 0), stop=(ec == ET - 1))
                            nc.tensor.matmul(pqi[:], Wt["qi"][:, ec * 512 + mt * P: ec * 512 + (mt + 1) * P],
                                             xq_i[:, ec * CH:(ec + 1) * CH],
                                             start=(ec == 0), stop=(ec == ET - 1))
                        nc.scalar.activation(qr_st[:, mt * CH:(mt + 1) * CH], pqr[:],
                                             AF.Identity, bias=bias_t["qr"][:, mt:mt + 1])
                        nc.scalar.activation(qi_st[:, mt * CH:(mt + 1) * CH], pqi[:],
                                             AF.Identity, bias=bias_t["qi"][:, mt:mt + 1])
                    stage_to_cat(Qcat_d, qr_st, qi_st, c * CH)
                    for mt in range(4):
                        pkr = pps.tile([P, CH], F32, tag="pp", name="pkr")
                        pki = pps.tile([P, CH], F32, tag="pp", name="pki")
                        for ec in range(ET):
                            nc.tensor.matmul(pkr[:], Wt["kr"][:, ec * 512 + mt * P: ec * 512 + (mt + 1) * P],
                                             xk_r[:, ec * CH:(ec + 1) * CH],
                                             start=(ec == 0), stop=(ec == ET - 1))
                            nc.tensor.matmul(pki[:], Wt["ki"][:, ec * 512 + mt * P: ec * 512 + (mt + 1) * P],
                                             xk_i[:, ec * CH:(ec + 1) * CH],
                                             start=(ec == 0), stop=(ec == ET - 1))
                        nc.scalar.activation(kr_st[:, mt * CH:(mt + 1) * CH], pkr[:],
                                             AF.Identity, bias=bias_t["kr"][:, mt:mt + 1])
                        nc.scalar.activation(ki_st[:, mt * CH:(mt + 1) * CH], pki[:],
                                             AF.Identity, bias=bias_t["ki"][:, mt:mt + 1])
                    stage_to_cat(Kcat_d, kr_st, ki_st, c * CH)
                    for mi in (3 * c, 3 * c + 1, 3 * c + 2):
                        m0, msz = MTI[mi]
                        mr = m0 - c * CH
                        pvr = pps.tile([P, 512], F32, tag="pp", name="pvr")
                        pvi = pps.tile([P, 512], F32, tag="pp", name="pvi")
                        for ec in range(ET):
                            nc.tensor.matmul(pvr[0:msz, :], xk_r[:, ec * CH + mr: ec * CH + mr + msz],
                                             Wt["vr"][:, ec * 512:(ec + 1) * 512],
                                             start=(ec == 0), stop=(ec == ET - 1))
                            nc.tensor.matmul(pvi[0:msz, :], xk_i[:, ec * CH + mr: ec * CH + mr + msz],
                                             Wt["vi"][:, ec * 512:(ec + 1) * 512],
                                             start=(ec == 0), stop=(ec == ET - 1))
                        vco = Vc[mi][0:msz, :].rearrange("p (h c) -> p h c", h=NH)
                        nc.vector.tensor_add(
                            vco[:, :, 0:64],
                            pvr[0:msz, :].rearrange("p (h c) -> p h c", h=NH),
                            vbias["vr"][0:msz, :].rearrange("p (h c) -> p h c", h=NH))
                        nc.vector.tensor_add(
                            vco[:, :, 64:128],
                            pvi[0:msz, :].rearrange("p (h c) -> p h c", h=NH),
                            vbias["vi"][0:msz, :].rearrange("p (h c) -> p h c", h=NH))
                        nc.vector.memset(vco[:, :, 128:129], 1.0)

                gath_slabs = [None] * NCH
                for c in range(NCH):
                    fct, fst = cur_slab
                    xr = []
                    xi = []
                    for eb in range(ET):
                        pr = dps.tile([P, CH], F32, tag="pr", name="pr")
                        pi = dps.tile([P, CH], F32, tag="pi", name="pi")
                        for lc in range(LT):
                            stat = qns[:, lc * E + eb * P: lc * E + (eb + 1) * P]
                            nc.tensor.matmul(pr[:], stat, fct[:, lc * CH:(lc + 1) * CH],
                                             start=(lc == 0), stop=(lc == LT - 1))
                            nc.tensor.matmul(pi[:], stat, fst[:, lc * CH:(lc + 1) * CH],
                                             start=(lc == 0), stop=(lc == LT - 1))
                        xr_ = xfp.tile([P, CH], BF16, tag=f"xr{eb}", name=f"xr{eb}")
                        xi_ = xfp.tile([P, CH], BF16, tag=f"xi{eb}", name=f"xi{eb}")
                        # gamma folded in as per-partition scale (ones on kv cores)
                        nc.scalar.activation(xr_[:], pr[:], AF.Identity,
                                             scale=gam8[:, eb:eb + 1])
                        nc.scalar.activation(xi_[:], pi[:], AF.Identity,
                                             scale=gam8[:, eb:eb + 1])
                        if c == 0:
                            # beta contributes only to DC (f=0); zero on kv cores
                            nc.vector.scalar_tensor_tensor(
                                xr_[:, 0:1], bet8[:, eb:eb + 1], SQL,
                                xr_[:, 0:1],
                                op0=mybir.AluOpType.mult,
                                op1=mybir.AluOpType.add)
                        xr.append(xr_)
                        xi.append(xi_)
                    # spectra exchange for this chunk (gpsimd queue)
                    for eb in range(ET):
                        nc.gpsimd.dma_start(xfo_d[c].ap()[eb * P:(eb + 1) * P, :], xr[eb][:])
                        nc.gpsimd.dma_start(xfo_d[c].ap()[E + eb * P: E + (eb + 1) * P, :], xi[eb][:])
                    nc.gpsimd.collective_compute(
                        "AllGather",
                        mybir.AluOpType.bypass,
                        replica_groups=[[0, 1], [2, 3], [4, 5], [6, 7]],
                        ins=[xfo_d[c][:].opt()],
                        outs=[gath_d[c][:].opt()],
                    )
                    # load gathered slabs: q half rows [0,2E), kv half [2E,4E)
                    slabs = []
                    for half in range(4):  # q_r, q_i, k_r, k_i
                        sl = gslp.tile([P, ET * CH], BF16, tag=f"gs{half}", name=f"gs{half}")
                        nc.sync.dma_start(
                            sl[:].rearrange("p (g f) -> p g f", g=ET),
                            gath_d[c].ap()[half * E:(half + 1) * E, :].rearrange(
                                "(g p) f -> p g f", p=P))
                        slabs.append(sl)
                    gath_slabs[c] = slabs
                    if c + 2 < NCH:
                        nxt2_slab = load_fslab(fsl, c + 2)
                    else:
                        nxt2_slab = None
                    if c >= 1:
                        s = gath_slabs[c - 1]
                        proj_chunk(c - 1, (s[0], s[1]), (s[2], s[3]))
                    cur_slab, nxt_slab = nxt_slab, nxt2_slab
                s = gath_slabs[NCH - 1]
                proj_chunk(NCH - 1, (s[0], s[1]), (s[2], s[3]))

            # =================== attention + iDFT + Wo ===================
            with tc.tile_pool(name="catp", bufs=1) as catp, \
                 tc.tile_pool(name="oacc", bufs=1) as oacc, \
                 tc.tile_pool(name="gidft", bufs=1) as gidft:
                Qcat = catp.tile([P, NH * FP], BF16)
                Kcat = catp.tile([P, NH * FP], BF16)
                nc.sync.dma_start(Qcat[:], Qcat_d.ap())
                nc.sync.dma_start(Kcat[:], Kcat_d.ap())
                our = []
                oui = []
                for ti in range(len(FTI)):
                    our.append(oacc.tile([P, 512], BF16, tag=f"our{ti}", name=f"our{ti}"))
                    oui.append(oacc.tile([P, 512], BF16, tag=f"oui{ti}", name=f"oui{ti}"))
                # prefetch iDFT matrices
                Gc = []
                Gs = []
                for ti, (m0, msz) in enumerate(FTI):
                    gc = gidft.tile([P, L], BF16, tag=f"gc{ti}", name=f"gc{ti}")
                    gs = gidft.tile([P, L], BF16, tag=f"gs{ti}", name=f"gs{ti}")
                    nc.sync.dma_start(gc[0:msz, :], GcT_d.ap()[m0:m0 + msz, :])
                    nc.sync.dma_start(gs[0:msz, :], GsT_d.ap()[m0:m0 + msz, :])
                    Gc.append(gc)
                    Gs.append(gs)

                with tc.tile_pool(name="expp", bufs=2) as expp, \
                     tc.tile_pool(name="sps", bufs=2, space="PSUM") as sps, \
                     tc.tile_pool(name="avps", bufs=2, space="PSUM") as avps, \
                     tc.tile_pool(name="nrm", bufs=4) as nrm:
                    for h in range(NH):
                        ets = []
                        for mi, (m0, msz) in enumerate(MTI):
                            ps = sps.tile([P, 1536], F32, tag="sc", name="sc")
                            for (s0, ssz) in ((0, 512), (512, 512), (1024, 2)):
                                nc.tensor.matmul(ps[0:msz, s0:s0 + ssz],
                                                 Kcat[:, h * FP + m0: h * FP + m0 + msz],
                                                 Qcat[:, h * FP + s0: h * FP + s0 + ssz],
                                                 start=True, stop=True)
                            et = expp.tile([P, FP], BF16, tag=f"e{mi}", name=f"e{mi}")
                            nc.scalar.activation(et[0:msz, :], ps[0:msz, 0:FP],
                                                 AF.Exp, scale=float(D ** -0.5))
                            ets.append(et)
                        for ti, (l0, lsz) in enumerate(FTI):
                            av = avps.tile([P, 129], F32, tag="av", name="av")
                            n = len(MTI)
                            for mi, (m0, msz) in enumerate(MTI):
                                nc.tensor.matmul(av[0:lsz, :], ets[mi][0:msz, l0:l0 + lsz],
                                                 Vc[mi][0:msz, h * 129:(h + 1) * 129],
                                                 start=(mi == 0), stop=(mi == n - 1))
                            rcp = nrm.tile([P, 1], F32, tag="rcp", name="rcp")
                            nc.vector.reciprocal(rcp[0:lsz, :], av[0:lsz, 128:129])
                            nc.vector.tensor_scalar_mul(our[ti][0:lsz, h * 64:(h + 1) * 64],
                                                        av[0:lsz, 0:64], rcp[0:lsz, :])
                            nc.vector.tensor_scalar_mul(oui[ti][0:lsz, h * 64:(h + 1) * 64],
                                                        av[0:lsz, 64:128], rcp[0:lsz, :])

                # ---------------- iDFT ----------------
                with tc.tile_pool(name="ottp", bufs=1) as ottp, \
                     tc.tile_pool(name="ops", bufs=4, space="PSUM") as ops:
                    OTT = [ottp.tile([P, L], BF16, tag=f"OTT{i}", name=f"OTT{i}")
                           for i in range(4)]
                    n = len(FTI)
                    for tcp in range(2):
                        for e4 in range(4):
                            for t2 in range(2):
                                tq = (tcp * 2 + t2) * 512
                                po = ops.tile([P, 512], F32, tag="po", name="po")
                                for ti, (m0, msz) in enumerate(FTI):
                                    nc.tensor.matmul(po[:], our[ti][0:msz, e4 * P:(e4 + 1) * P],
                                                     Gc[ti][0:msz, tq:tq + 512],
                                                     start=(ti == 0), stop=False)
                                    nc.tensor.matmul(po[:], oui[ti][0:msz, e4 * P:(e4 + 1) * P],
                                                     Gs[ti][0:msz, tq:tq + 512],
                                                     start=False, stop=(ti == n - 1))
                                nc.scalar.activation(OTT[e4][:, tq:tq + 512], po[:], AF.Copy)

                    # ---------------- Wo ----------------
                    with tc.tile_pool(name="wop", bufs=1) as wop, \
                         tc.tile_pool(name="ost", bufs=3) as ost, \
                         tc.tile_pool(name="wops", bufs=4, space="PSUM") as wops:
                        WoT_t = wop.tile([P, 4 * E], BF16)
                        nc.sync.dma_start(WoT_t[:].rearrange("p (c e) -> p c e", c=4),
                                          WoT_d.ap().rearrange("(c p) e -> p c e", p=P))
                        for tb in range(LT):
                            ot_ = ost.tile([P, E], F32, tag="ot", name="ot")
                            for eo in range(2):
                                pso = wops.tile([P, 512], F32, tag="po2", name="pso")
                                for ec in range(4):
                                    nc.tensor.matmul(pso[:],
                                                     OTT[ec][:, tb * P:(tb + 1) * P],
                                                     WoT_t[:, ec * E + eo * 512: ec * E + (eo + 1) * 512],
                                                     start=(ec == 0), stop=(ec == 3))
                                if eo == 0:
                                    nc.scalar.activation(ot_[:, 0:512], pso[:], AF.Copy)
                                else:
                                    nc.vector.tensor_copy(ot_[:, 512:1024], pso[:])
                            nc.sync.dma_start(out_d.ap()[tb * P:(tb + 1) * P, :], ot_[:])

    nc.finalize()
    return nc


def kernel(**inputs):
    from concourse.bass_utils import run_bass_kernel_spmd

    if "nc" not in _CACHE:
        _CACHE["nc"] = _build()
        _CACHE["consts"] = _dft_consts()
    nc = _CACHE["nc"]
    FcT, FsT, GcT, GsT = _CACHE["consts"]

    rdt = ml_dtypes.bfloat16
    q = np.ascontiguousarray(inputs["query"], dtype=rdt)
    kv = np.ascontiguousarray(inputs["key_value"], dtype=rdt)
    gamma = np.ascontiguousarray(inputs["gamma"], np.float32).reshape(E, 1)
    beta = np.ascontiguousarray(inputs["beta"], np.float32).reshape(E, 1)
    ones_g = np.ones((E, 1), np.float32)
    zeros_b = np.zeros((E, 1), np.float32)
    in_maps = []
    for core in range(8):
        b = core // 2
        hg = core % 2
        is_q = (core % 2 == 0)
        cs = slice(hg * 512, (hg + 1) * 512)
        m = {
            "x": q[b] if is_q else np.ascontiguousarray(kv[b]),
            "lnm": np.full((P, 1), 1.0 if is_q else 0.0, np.float32),
            "gamma": gamma if is_q else ones_g,
            "beta": beta if is_q else zeros_b,
            "FcT": FcT.astype(rdt), "FsT": FsT.astype(rdt),
            "GcT": GcT.astype(rdt), "GsT": GsT.astype(rdt),
            "WoT": np.ascontiguousarray(inputs["Wo"][:, cs].T.astype(rdt)),
        }
        for nm in ("qr", "qi", "kr", "ki", "vr", "vi"):
            m[f"W{nm}"] = np.ascontiguousarray(inputs["W" + nm][cs, :].T.astype(rdt))
            m[f"b{nm}"] = np.ascontiguousarray(inputs["b" + nm][cs], np.float32).reshape(512, 1)
        in_maps.append(m)

    res = run_bass_kernel_spmd(nc, in_maps, core_ids=list(range(8)))
    _CACHE["last"] = res
    out = np.empty((B, L, E), np.float32)
    for b in range(B):
        out[b] = res.results[2 * b]["out"] + res.results[2 * b + 1]["out"]
    return out


# revision 34
# speedup vs baseline: 1.0091x; 1.0091x over previous
"""Trainium2 Bass kernel for nn_FreqCrossAttention.

Sharding: 8 cores = 4 batches x 2 head-groups (8 heads each), with the
DFT work split across each batch pair: the even core computes the DFT of
LN(query), the odd core the DFT of key_value (LN is switched off via a
per-core mask input so the program stays uniform). The two spectra are
exchanged with a pairwise chunked AllGather, after which both cores run
the 6 projections for their own head-group, attention, iDFT and the
row-parallel W_o partial. Host sums the pair per batch.

All matmuls bf16; everything SBUF-resident except the Q/K staging and
the spectra exchange, which go through DRAM.
"""
import math
import numpy as np
import ml_dtypes

B, L, E, H = 4, 2048, 1024, 16
D = E // H            # 64
Lf = L // 2 + 1       # 1025
FP = 1026             # padded frequency dim (= 3*342)
NH = 8                # heads per core
P = 128
CH = 342              # DFT/proj frequency chunk
NCH = 3
LT = 16               # L tiles of 128
ET = 8                # e-chunks of E
EPS = 1e-5
SQL = math.sqrt(L)
# m-tiles (keys), chunk-aligned: per chunk (0,128),(128,128),(256,86);
# the final tile is 85 rows (m=940..1024) so the pad key row 1025 is
# never part of the softmax.
MTI = []
for _c in range(NCH):
    last = 85 if _c == NCH - 1 else 86
    MTI += [(_c * CH, 128), (_c * CH + 128, 128), (_c * CH + 256, last)]
# l-tiles (queries/outputs) incl pad col 1025
FTI = [(i * P, P) for i in range(8)] + [(1024, 2)]

_CACHE = {}


def _dft_consts():
    f = np.arange(FP)
    t = np.arange(L)
    ang = 2.0 * np.pi * np.outer(t, f) / L            # [L, FP]
    s = 1.0 / math.sqrt(L)
    FcT = (np.cos(ang) * s).astype(np.float32)        # rhs for rfft [L, FP]
    FsT = (-np.sin(ang) * s).astype(np.float32)
    FcT[:, Lf:] = 0.0
    FsT[:, Lf:] = 0.0
    cw = np.where((f == 0) | (f == L // 2), 1.0, 2.0)[:, None]
    GcT = (cw * np.cos(ang.T) * s).astype(np.float32)  # [FP, L]
    GsT = (-cw * np.sin(ang.T) * s).astype(np.float32)
    GcT[Lf:, :] = 0.0
    GsT[Lf:, :] = 0.0
    return FcT, FsT, GcT, GsT


def _build():
    import concourse.bass as bass
    import concourse.bacc as bacc
    import concourse.mybir as mybir
    import concourse.tile as tile

    F32 = mybir.dt.float32
    BF16 = mybir.dt.bfloat16
    AF = mybir.ActivationFunctionType

    nc = bacc.Bacc("TRN2", debug=False, num_devices=8)

    x_d = nc.dram_tensor("x", [L, E], BF16, kind="ExternalInput")
    lnm_d = nc.dram_tensor("lnm", [P, 1], F32, kind="ExternalInput")
    gamma_d = nc.dram_tensor("gamma", [E, 1], F32, kind="ExternalInput")
    beta_d = nc.dram_tensor("beta", [E, 1], F32, kind="ExternalInput")
    FcT_d = nc.dram_tensor("FcT", [L, FP], BF16, kind="ExternalInput")
    FsT_d = nc.dram_tensor("FsT", [L, FP], BF16, kind="ExternalInput")
    GcT_d = nc.dram_tensor("GcT", [FP, L], BF16, kind="ExternalInput")
    GsT_d = nc.dram_tensor("GsT", [FP, L], BF16, kind="ExternalInput")
    W_d = {}
    for nm in ("qr", "qi", "kr", "ki", "vr", "vi"):
        W_d[nm] = nc.dram_tensor(f"W{nm}", [E, 512], BF16, kind="ExternalInput")
        W_d["b" + nm] = nc.dram_tensor(f"b{nm}", [512, 1], F32, kind="ExternalInput")
    WoT_d = nc.dram_tensor("WoT", [512, E], BF16, kind="ExternalInput")
    out_d = nc.dram_tensor("out", [L, E], F32, kind="ExternalOutput")
    # spectra exchange buffers: rows = ri*1024 + eb*128 + p
    xfo_d = [nc.dram_tensor(f"xfo{c}", [2 * E, CH], BF16) for c in range(NCH)]
    gath_d = [nc.dram_tensor(f"gath{c}", [4 * E, CH], BF16) for c in range(NCH)]
    # Q/K staging (head-interleaved cat layout, rows 0:64 real / 64:128 imag)
    Qcat_d = nc.dram_tensor("Qcat", [P, NH * FP], BF16)
    Kcat_d = nc.dram_tensor("Kcat", [P, NH * FP], BF16)

    with tile.TileContext(nc) as tc:
        with tc.tile_pool(name="persist", bufs=1) as persist:
            Vc = [persist.tile([P, NH * 129], BF16, tag=f"Vc{i}", name=f"Vc{i}")
                  for i in range(len(MTI))]
            # constants (gpsimd DMA queue keeps sync free for x/fct loads)
            eps_t = persist.tile([P, 1], F32)
            nc.vector.memset(eps_t[:], EPS)
            lnm = persist.tile([P, 1], F32)
            nc.gpsimd.dma_start(lnm[:], lnm_d.ap())
            gam8 = persist.tile([P, ET], F32)
            nc.gpsimd.dma_start(gam8[:], gamma_d.ap().rearrange("(c p) one -> p (c one)", p=P))
            bet8 = persist.tile([P, ET], F32)
            nc.gpsimd.dma_start(bet8[:], beta_d.ap().rearrange("(c p) one -> p (c one)", p=P))
            bias_t = {}
            for nm in ("qr", "qi", "kr", "ki"):
                bias_t[nm] = persist.tile([P, 4], F32, tag=f"b{nm}", name=f"b{nm}")
                nc.gpsimd.dma_start(bias_t[nm][:],
                                    W_d["b" + nm].ap().rearrange("(mt p) one -> p (mt one)", p=P))

            def load_w(names, wp):
                Wt = {}
                for nm in names:
                    Wt[nm] = wp.tile([P, ET * 512], BF16, tag=f"W{nm}", name=f"W{nm}")
                    nc.gpsimd.dma_start(
                        Wt[nm][:].rearrange("p (c e) -> p c e", c=ET),
                        W_d[nm].ap().rearrange("(c p) e -> p c e", p=P))
                return Wt

            def load_fslab(fsl, c):
                # split in lc-halves so the first DFT matmuls can start early
                f0 = c * CH
                fct = fsl.tile([P, LT * CH], BF16, tag="fct", name="fct")
                fst = fsl.tile([P, LT * CH], BF16, tag="fst", name="fst")
                hl = LT // 2
                for hf in range(2):
                    cs_ = slice(hf * hl * CH, (hf + 1) * hl * CH)
                    rs_ = slice(hf * hl * P, (hf + 1) * hl * P)
                    nc.sync.dma_start(
                        fct[:, cs_].rearrange("p (lc f) -> p lc f", lc=hl),
                        FcT_d.ap()[rs_, f0:f0 + CH].rearrange("(lc p) f -> p lc f", p=P))
                    nc.sync.dma_start(
                        fst[:, cs_].rearrange("p (lc f) -> p lc f", lc=hl),
                        FsT_d.ap()[rs_, f0:f0 + CH].rearrange("(lc p) f -> p lc f", p=P))
                return fct, fst

            def stage_to_cat(cat_d, stage_r, stage_i, f0):
                # heads h = 2*mt + hh ; real rows 0:64 of cat, imag rows 64:128
                # (scalar DMA queue: keep gpsimd free for collective triggers)
                catv = cat_d.ap().rearrange("p (h f) -> p h f", h=NH)
                for hh in range(2):
                    nc.scalar.dma_start(
                        catv[0:64, hh::2, f0:f0 + CH],
                        stage_r[hh * 64:(hh + 1) * 64, :].rearrange("p (mt f) -> p mt f", f=CH))
                    nc.scalar.dma_start(
                        catv[64:128, hh::2, f0:f0 + CH],
                        stage_i[hh * 64:(hh + 1) * 64, :].rearrange("p (mt f) -> p mt f", f=CH))

            # =================== DFT phase (one path per core) ===================
            with tc.tile_pool(name="qnsp", bufs=1) as qnsp, \
                 tc.tile_pool(name="qin", bufs=2) as qin, \
                 tc.tile_pool(name="lns", bufs=4) as lns, \
                 tc.tile_pool(name="wpool", bufs=1) as wpool, \
                 tc.tile_pool(name="fsl", bufs=1) as fsl, \
                 tc.tile_pool(name="xfp", bufs=1) as xfp, \
                 tc.tile_pool(name="gsl", bufs=2) as gslp, \
                 tc.tile_pool(name="stg", bufs=1) as stg, \
                 tc.tile_pool(name="dps", bufs=2, space="PSUM") as dps, \
                 tc.tile_pool(name="pps", bufs=4, space="PSUM") as pps:
                qns = qnsp.tile([P, LT * E], BF16)

                # ---- masked LN (x streams first on sync; DVE stats, ACT apply) ----
                # mask helpers: lnm1 = 1-m, lnmn = -m
                lnm1 = persist.tile([P, 1], F32)
                nc.vector.tensor_scalar(lnm1[:], lnm[:], -1.0, 1.0,
                                        op0=mybir.AluOpType.mult,
                                        op1=mybir.AluOpType.add)
                lnmn = persist.tile([P, 1], F32)
                nc.vector.tensor_scalar_mul(lnmn[:], lnm[:], -1.0)
                for qq in range(8):
                    qsl = qin.tile([P, 2 * E], BF16, tag="qsl", name="qsl")
                    nc.sync.dma_start(
                        qsl[:].rearrange("p (lc e) -> p lc e", lc=2),
                        x_d.ap()[qq * 2 * P:(qq + 1) * 2 * P, :].rearrange(
                            "(lc p) e -> p lc e", p=P))
                    for lq in range(2):
                        lc = qq * 2 + lq
                        qt = qsl[:, lq * E:(lq + 1) * E]
                        st = lns.tile([P, 12], F32, tag="st", name="st")
                        nc.vector.bn_stats(st[:, 0:6], qt[:, 0:512])
                        nc.vector.bn_stats(st[:, 6:12], qt[:, 512:1024])
                        mv = lns.tile([P, 2], F32, tag="mv", name="mv")
                        nc.vector.bn_aggr(mv[:], st[:])
                        sd = lns.tile([P, 1], F32, tag="sd", name="sd")
                        nc.scalar.activation(sd[:], mv[:, 1:2], AF.Sqrt, bias=eps_t[:])
                        istd = lns.tile([P, 1], F32, tag="istd", name="istd")
                        nc.vector.reciprocal(istd[:], sd[:])
                        # masked scale = m*istd + (1-m); masked bias = -m*mu*istd
                        nc.vector.tensor_scalar(istd[:], istd[:], lnm[:], lnm1[:],
                                                op0=mybir.AluOpType.mult,
                                                op1=mybir.AluOpType.add)
                        nmu = lns.tile([P, 1], F32, tag="nmu", name="nmu")
                        nc.vector.tensor_scalar(nmu[:], mv[:, 0:1], istd[:], lnmn[:],
                                                op0=mybir.AluOpType.mult,
                                                op1=mybir.AluOpType.mult)
                        nc.scalar.activation(qns[:, lc * E:(lc + 1) * E], qt, AF.Identity,
                                             bias=nmu[:], scale=istd[:])

                vbias = {}
                for nm in ("vr", "vi"):
                    vb_row = qin.tile([1, 512], F32, tag="qsl", name="vb_row")
                    nc.gpsimd.dma_start(vb_row[:], W_d["b" + nm].ap().rearrange("e one -> one e"))
                    vb = wpool.tile([P, 512], F32, tag=f"vb{nm}", name=f"vb{nm}")
                    nc.gpsimd.partition_broadcast(vb[:], vb_row[:])
                    vbias[nm] = vb
                Wt = load_w(("kr", "ki", "vr", "vi", "qr", "qi"), wpool)

                cur_slab = load_fslab(fsl, 0)
                nxt_slab = load_fslab(fsl, 1)

                # ---- DFT chunks + pipelined exchange + projections ----
                def proj_chunk(c, xq, gk):
                    """Q/K/V projections for chunk c. xq: [128,(8 eb)*CH] q-spectrum
                    slab (real, imag); gk: kv-spectrum slabs."""
                    xq_r, xq_i = xq
                    xk_r, xk_i = gk
                    qr_st = stg.tile([P, 4 * CH], BF16, tag="qr_st", name="qr_st")
                    qi_st = stg.tile([P, 4 * CH], BF16, tag="qi_st", name="qi_st")
                    kr_st = stg.tile([P, 4 * CH], BF16, tag="kr_st", name="kr_st")
                    ki_st = stg.tile([P, 4 * CH], BF16, tag="ki_st", name="ki_st")
                    for mt in range(4):
                        pqr = pps.tile([P, CH], F32, tag="pp", name="pqr")
                        pqi = pps.tile([P, CH], F32, tag="pp", name="pqi")
                        for ec in range(ET):
                            nc.tensor.matmul(pqr[:], Wt["qr"][:, ec * 512 + mt * P: ec * 512 + (mt + 1) * P],
                                             xq_r[:, ec * CH:(ec + 1) * CH],
                                             start=(ec == 0), stop=(ec == ET - 1))
                            nc.tensor.matmul(pqi[:], Wt["qi"][:, ec * 512 + mt * P: ec * 512 + (mt + 1) * P],
                                             xq_i[:, ec * CH:(ec + 1) * CH],
                                             start=(ec == 0), stop=(ec == ET - 1))
                        nc.scalar.activation(qr_st[:, mt * CH:(mt + 1) * CH], pqr[:],
                                             AF.Identity, bias=bias_t["qr"][:, mt:mt + 1])
                        nc.scalar.activation(qi_st[:, mt * CH:(mt + 1) * CH], pqi[:],
                                             AF.Identity, bias=bias_t["qi"][:, mt:mt + 1])
                    stage_to_cat(Qcat_d, qr_st, qi_st, c * CH)
                    for mt in range(4):
                        pkr = pps.tile([P, CH], F32, tag="pp", name="pkr")
                        pki = pps.tile([P, CH], F32, tag="pp", name="pki")
                        for ec in range(ET):
                            nc.tensor.matmul(pkr[:], Wt["kr"][:, ec * 512 + mt * P: ec * 512 + (mt + 1) * P],
                                             xk_r[:, ec * CH:(ec + 1) * CH],
                                             start=(ec == 0), stop=(ec == ET - 1))
                            nc.tensor.matmul(pki[:], Wt["ki"][:, ec * 512 + mt * P: ec * 512 + (mt + 1) * P],
                                             xk_i[:, ec * CH:(ec + 1) * CH],
                                             start=(ec == 0), stop=(ec == ET - 1))
                        nc.scalar.activation(kr_st[:, mt * CH:(mt + 1) * CH], pkr[:],
                                             AF.Identity, bias=bias_t["kr"][:, mt:mt + 1])
                        nc.scalar.activation(ki_st[:, mt * CH:(mt + 1) * CH], pki[:],
                                             AF.Identity, bias=bias_t["ki"][:, mt:mt + 1])
                    stage_to_cat(Kcat_d, kr_st, ki_st, c * CH)
                    for mi in (3 * c, 3 * c + 1, 3 * c + 2):
                        m0, msz = MTI[mi]
                        mr = m0 - c * CH
                        pvr = pps.tile([P, 512], F32, tag="pp", name="pvr")
                        pvi = pps.tile([P, 512], F32, tag="pp", name="pvi")
                        for ec in range(ET):
                            nc.tensor.matmul(pvr[0:msz, :], xk_r[:, ec * CH + mr: ec * CH + mr + msz],
                                             Wt["vr"][:, ec * 512:(ec + 1) * 512],
                                             start=(ec == 0), stop=(ec == ET - 1))
                            nc.tensor.matmul(pvi[0:msz, :], xk_i[:, ec * CH + mr: ec * CH + mr + msz],
                                             Wt["vi"][:, ec * 512:(ec + 1) * 512],
                                             start=(ec == 0), stop=(ec == ET - 1))
                        vco = Vc[mi][0:msz, :].rearrange("p (h c) -> p h c", h=NH)
                        nc.vector.tensor_add(
                            vco[:, :, 0:64],
                            pvr[0:msz, :].rearrange("p (h c) -> p h c", h=NH),
                            vbias["vr"][0:msz, :].rearrange("p (h c) -> p h c", h=NH))
                        nc.vector.tensor_add(
                            vco[:, :, 64:128],
                            pvi[0:msz, :].rearrange("p (h c) -> p h c", h=NH),
                            vbias["vi"][0:msz, :].rearrange("p (h c) -> p h c", h=NH))
                        nc.vector.memset(vco[:, :, 128:129], 1.0)

                gath_slabs = [None] * NCH
                for c in range(NCH):
                    fct, fst = cur_slab
                    xr = [None] * ET
                    xi = [None] * ET
                    # lc-outer accumulation in 2-eb groups: the first chunk's
                    # matmuls chase LN tile-by-tile instead of waiting for the
                    # full qns slab
                    for g in range(ET // 2):
                        ebs = (2 * g, 2 * g + 1)
                        prs = {}
                        pis = {}
                        for eb in ebs:
                            prs[eb] = dps.tile([P, CH], F32, tag="pr", name="pr")
                            pis[eb] = dps.tile([P, CH], F32, tag="pi", name="pi")
                        for lc in range(LT):
                            for eb in ebs:
                                stat = qns[:, lc * E + eb * P: lc * E + (eb + 1) * P]
                                nc.tensor.matmul(prs[eb][:], stat, fct[:, lc * CH:(lc + 1) * CH],
                                                 start=(lc == 0), stop=(lc == LT - 1))
                                nc.tensor.matmul(pis[eb][:], stat, fst[:, lc * CH:(lc + 1) * CH],
                                                 start=(lc == 0), stop=(lc == LT - 1))
                        for eb in ebs:
                            xr_ = xfp.tile([P, CH], BF16, tag=f"xr{eb}", name=f"xr{eb}")
                            xi_ = xfp.tile([P, CH], BF16, tag=f"xi{eb}", name=f"xi{eb}")
                            # gamma folded in as per-partition scale (ones on kv cores)
                            nc.scalar.activation(xr_[:], prs[eb][:], AF.Identity,
                                                 scale=gam8[:, eb:eb + 1])
                            nc.scalar.activation(xi_[:], pis[eb][:], AF.Identity,
                                                 scale=gam8[:, eb:eb + 1])
                            if c == 0:
                                # beta contributes only to DC (f=0); zero on kv cores
                                nc.vector.scalar_tensor_tensor(
                                    xr_[:, 0:1], bet8[:, eb:eb + 1], SQL,
                                    xr_[:, 0:1],
                                    op0=mybir.AluOpType.mult,
                                    op1=mybir.AluOpType.add)
                            xr[eb] = xr_
                            xi[eb] = xi_
                            # spectra exchange writes (sync queue)
                            nc.sync.dma_start(xfo_d[c].ap()[eb * P:(eb + 1) * P, :], xr_[:])
                            nc.sync.dma_start(xfo_d[c].ap()[E + eb * P: E + (eb + 1) * P, :], xi_[:])
                    nc.gpsimd.collective_compute(
                        "AllGather",
                        mybir.AluOpType.bypass,
                        replica_groups=[[0, 1], [2, 3], [4, 5], [6, 7]],
                        ins=[xfo_d[c][:].opt()],
                        outs=[gath_d[c][:].opt()],
                    )
                    # load gathered slabs: q half rows [0,2E), kv half [2E,4E)
                    slabs = []
                    for half in range(4):  # q_r, q_i, k_r, k_i
                        sl = gslp.tile([P, ET * CH], BF16, tag=f"gs{half}", name=f"gs{half}")
                        nc.sync.dma_start(
                            sl[:].rearrange("p (g f) -> p g f", g=ET),
                            gath_d[c].ap()[half * E:(half + 1) * E, :].rearrange(
                                "(g p) f -> p g f", p=P))
                        slabs.append(sl)
                    gath_slabs[c] = slabs
                    if c + 2 < NCH:
                        nxt2_slab = load_fslab(fsl, c + 2)
                    else:
                        nxt2_slab = None
                    if c >= 1:
                        s = gath_slabs[c - 1]
                        proj_chunk(c - 1, (s[0], s[1]), (s[2], s[3]))
                    cur_slab, nxt_slab = nxt_slab, nxt2_slab
                s = gath_slabs[NCH - 1]
                proj_chunk(NCH - 1, (s[0], s[1]), (s[2], s[3]))

            # =================== attention + iDFT + Wo ===================
            with tc.tile_pool(name="catp", bufs=1) as catp, \
                 tc.tile_pool(name="oacc", bufs=1) as oacc, \
                 tc.tile_pool(name="gidft", bufs=1) as gidft:
                Qcat = catp.tile([P, NH * FP], BF16)
                Kcat = catp.tile([P, NH * FP], BF16)
                for hp in range(4):
                    cs_ = slice(hp * 2 * FP, (hp + 1) * 2 * FP)
                    nc.sync.dma_start(Qcat[:, cs_], Qcat_d.ap()[:, cs_])
                    nc.sync.dma_start(Kcat[:, cs_], Kcat_d.ap()[:, cs_])
                our = []
                oui = []
                for ti in range(len(FTI)):
                    our.append(oacc.tile([P, 512], BF16, tag=f"our{ti}", name=f"our{ti}"))
                    oui.append(oacc.tile([P, 512], BF16, tag=f"oui{ti}", name=f"oui{ti}"))
                # prefetch iDFT matrices
                Gc = []
                Gs = []
                for ti, (m0, msz) in enumerate(FTI):
                    gc = gidft.tile([P, L], BF16, tag=f"gc{ti}", name=f"gc{ti}")
                    gs = gidft.tile([P, L], BF16, tag=f"gs{ti}", name=f"gs{ti}")
                    nc.sync.dma_start(gc[0:msz, :], GcT_d.ap()[m0:m0 + msz, :])
                    nc.sync.dma_start(gs[0:msz, :], GsT_d.ap()[m0:m0 + msz, :])
                    Gc.append(gc)
                    Gs.append(gs)

                with tc.tile_pool(name="expp", bufs=2) as expp, \
                     tc.tile_pool(name="sps", bufs=2, space="PSUM") as sps, \
                     tc.tile_pool(name="avps", bufs=2, space="PSUM") as avps, \
                     tc.tile_pool(name="nrm", bufs=4) as nrm:
                    for h in range(NH):
                        ets = []
                        for mi, (m0, msz) in enumerate(MTI):
                            ps = sps.tile([P, 1536], F32, tag="sc", name="sc")
                            for (s0, ssz) in ((0, 512), (512, 512), (1024, 2)):
                                nc.tensor.matmul(ps[0:msz, s0:s0 + ssz],
                                                 Kcat[:, h * FP + m0: h * FP + m0 + msz],
                                                 Qcat[:, h * FP + s0: h * FP + s0 + ssz],
                                                 start=True, stop=True)
                            et = expp.tile([P, FP], BF16, tag=f"e{mi}", name=f"e{mi}")
                            nc.scalar.activation(et[0:msz, :], ps[0:msz, 0:FP],
                                                 AF.Exp, scale=float(D ** -0.5))
                            ets.append(et)
                        for ti, (l0, lsz) in enumerate(FTI):
                            av = avps.tile([P, 129], F32, tag="av", name="av")
                            n = len(MTI)
                            for mi, (m0, msz) in enumerate(MTI):
                                nc.tensor.matmul(av[0:lsz, :], ets[mi][0:msz, l0:l0 + lsz],
                                                 Vc[mi][0:msz, h * 129:(h + 1) * 129],
                                                 start=(mi == 0), stop=(mi == n - 1))
                            rcp = nrm.tile([P, 1], F32, tag="rcp", name="rcp")
                            nc.vector.reciprocal(rcp[0:lsz, :], av[0:lsz, 128:129])
                            nc.vector.tensor_scalar_mul(our[ti][0:lsz, h * 64:(h + 1) * 64],
                                                        av[0:lsz, 0:64], rcp[0:lsz, :])
                            nc.vector.tensor_scalar_mul(oui[ti][0:lsz, h * 64:(h + 1) * 64],
                                                        av[0:lsz, 64:128], rcp[0:lsz, :])

                # ---------------- iDFT ----------------
                with tc.tile_pool(name="ottp", bufs=1) as ottp, \
                     tc.tile_pool(name="ops", bufs=4, space="PSUM") as ops:
                    OTT = [ottp.tile([P, L], BF16, tag=f"OTT{i}", name=f"OTT{i}")
                           for i in range(4)]
                    n = len(FTI)
                    for tcp in range(2):
                        for e4 in range(4):
                            for t2 in range(2):
                                tq = (tcp * 2 + t2) * 512
                                po = ops.tile([P, 512], F32, tag="po", name="po")
                                for ti, (m0, msz) in enumerate(FTI):
                                    nc.tensor.matmul(po[:], our[ti][0:msz, e4 * P:(e4 + 1) * P],
                                                     Gc[ti][0:msz, tq:tq + 512],
                                                     start=(ti == 0), stop=False)
                                    nc.tensor.matmul(po[:], oui[ti][0:msz, e4 * P:(e4 + 1) * P],
                                                     Gs[ti][0:msz, tq:tq + 512],
                                                     start=False, stop=(ti == n - 1))
                                nc.scalar.activation(OTT[e4][:, tq:tq + 512], po[:], AF.Copy)

                    # ---------------- Wo ----------------
                    with tc.tile_pool(name="wop", bufs=1) as wop, \
                         tc.tile_pool(name="ost", bufs=3) as ost, \
                         tc.tile_pool(name="wops", bufs=4, space="PSUM") as wops:
                        WoT_t = wop.tile([P, 4 * E], BF16)
                        nc.sync.dma_start(WoT_t[:].rearrange("p (c e) -> p c e", c=4),
                                          WoT_d.ap().rearrange("(c p) e -> p c e", p=P))
                        for tb in range(LT):
                            ot_ = ost.tile([P, E], F32, tag="ot", name="ot")
                            for eo in range(2):
                                pso = wops.tile([P, 512], F32, tag="po2", name="pso")
                                for ec in range(4):
                                    nc.tensor.matmul(pso[:],
                                                     OTT[ec][:, tb * P:(tb + 1) * P],
                                                     WoT_t[:, ec * E + eo * 512: ec * E + (eo + 1) * 512],
                                                     start=(ec == 0), stop=(ec == 3))
                                if eo == 0:
                                    nc.scalar.activation(ot_[:, 0:512], pso[:], AF.Copy)
                                else:
                                    nc.vector.tensor_copy(ot_[:, 512:1024], pso[:])
                            nc.sync.dma_start(out_d.ap()[tb * P:(tb + 1) * P, :], ot_[:])

    nc.finalize()
    return nc


def kernel(**inputs):
    from concourse.bass_utils import run_bass_kernel_spmd

    if "nc" not in _CACHE:
        _CACHE["nc"] = _build()
        _CACHE["consts"] = _dft_consts()
    nc = _CACHE["nc"]
    FcT, FsT, GcT, GsT = _CACHE["consts"]

    rdt = ml_dtypes.bfloat16
    q = np.ascontiguousarray(inputs["query"], dtype=rdt)
    kv = np.ascontiguousarray(inputs["key_value"], dtype=rdt)
    gamma = np.ascontiguousarray(inputs["gamma"], np.float32).reshape(E, 1)
    beta = np.ascontiguousarray(inputs["beta"], np.float32).reshape(E, 1)
    ones_g = np.ones((E, 1), np.float32)
    zeros_b = np.zeros((E, 1), np.float32)
    in_maps = []
    for core in range(8):
        b = core // 2
        hg = core % 2
        is_q = (core % 2 == 0)
        cs = slice(hg * 512, (hg + 1) * 512)
        m = {
            "x": q[b] if is_q else np.ascontiguousarray(kv[b]),
            "lnm": np.full((P, 1), 1.0 if is_q else 0.0, np.float32),
            "gamma": gamma if is_q else ones_g,
            "beta": beta if is_q else zeros_b,
            "FcT": FcT.astype(rdt), "FsT": FsT.astype(rdt),
            "GcT": GcT.astype(rdt), "GsT": GsT.astype(rdt),
            "WoT": np.ascontiguousarray(inputs["Wo"][:, cs].T.astype(rdt)),
        }
        for nm in ("qr", "qi", "kr", "ki", "vr", "vi"):
            m[f"W{nm}"] = np.ascontiguousarray(inputs["W" + nm][cs, :].T.astype(rdt))
            m[f"b{nm}"] = np.ascontiguousarray(inputs["b" + nm][cs], np.float32).reshape(512, 1)
        in_maps.append(m)

    res = run_bass_kernel_spmd(nc, in_maps, core_ids=list(range(8)))
    _CACHE["last"] = res
    out = np.empty((B, L, E), np.float32)
    for b in range(B):
        out[b] = res.results[2 * b]["out"] + res.results[2 * b + 1]["out"]
    return out


# revision 37
# speedup vs baseline: 1.2206x; 1.2095x over previous
"""Trainium2 Bass kernel for nn_FreqCrossAttention.

Sharding: 8 cores = 4 batches x 2 head-groups (8 heads each).
Each core computes a partial output [2048, 1024] (its head-group's
contribution through W_o row-parallel); host sums the pair per batch.

v4: fully SBUF-resident pipeline (no DRAM round-trips for qn/Q/K/V),
slab DMA loads (2KB lines), LN overlapped under the kv-path DFT,
lc-outer DFT accumulation (matmuls chase the streaming input loads),
342-wide f-chunks (3 x 342 = 1026), chunk-aligned V/m-tiles, exp over
a 3-bank PSUM scores tile, iDFT computed for t=0..1024 only and
mirrored to the second half via cos/sin symmetry, all matmuls bf16.
"""
import math
import numpy as np
import ml_dtypes

B, L, E, H = 4, 2048, 1024, 16
D = E // H            # 64
Lf = L // 2 + 1       # 1025
FP = 1026             # padded frequency dim (= 3*342)
NH = 8                # heads per core
P = 128
CH = 342              # DFT/proj frequency chunk
NCH = 3
LT = 16               # L tiles of 128
ET = 8                # e-chunks of E
EPS = 1e-5
SQL = math.sqrt(L)
# m-tiles (keys), chunk-aligned: per chunk (0,128),(128,128),(256,86);
# the final tile is 85 rows (m=940..1024) so the pad key row 1025 is
# never part of the softmax.
MTI = []
for _c in range(NCH):
    last = 85 if _c == NCH - 1 else 86
    MTI += [(_c * CH, 128), (_c * CH + 128, 128), (_c * CH + 256, last)]
# l-tiles (queries/outputs) incl pad col 1025
FTI = [(i * P, P) for i in range(8)] + [(1024, 2)]
# iDFT half-range t-chunks (t = 0..1024); the rest mirrors
TCH = [(0, 342), (342, 342), (684, 341)]

_CACHE = {}


def _dft_consts():
    f = np.arange(FP)
    t = np.arange(L)
    ang = 2.0 * np.pi * np.outer(t, f) / L            # [L, FP]
    s = 1.0 / math.sqrt(L)
    FcT = (np.cos(ang) * s).astype(np.float32)        # rhs for rfft [L, FP]
    FsT = (-np.sin(ang) * s).astype(np.float32)
    FcT[:, Lf:] = 0.0
    FsT[:, Lf:] = 0.0
    cw = np.where((f == 0) | (f == L // 2), 1.0, 2.0)[:, None]
    GcT = (cw * np.cos(ang.T) * s).astype(np.float32)  # [FP, L]
    GsT = (-cw * np.sin(ang.T) * s).astype(np.float32)
    GcT[Lf:, :] = 0.0
    GsT[Lf:, :] = 0.0
    return FcT, FsT, GcT, GsT


def _build():
    import concourse.bass as bass
    import concourse.bacc as bacc
    import concourse.mybir as mybir
    import concourse.tile as tile

    F32 = mybir.dt.float32
    BF16 = mybir.dt.bfloat16
    AF = mybir.ActivationFunctionType

    nc = bacc.Bacc("TRN2", debug=False, num_devices=8)

    q_d = nc.dram_tensor("q", [L, E], BF16, kind="ExternalInput")
    kv_d = nc.dram_tensor("kv", [L, E], BF16, kind="ExternalInput")
    gamma_d = nc.dram_tensor("gamma", [E, 1], F32, kind="ExternalInput")
    beta_d = nc.dram_tensor("beta", [E, 1], F32, kind="ExternalInput")
    FcT_d = nc.dram_tensor("FcT", [L, FP], BF16, kind="ExternalInput")
    FsT_d = nc.dram_tensor("FsT", [L, FP], BF16, kind="ExternalInput")
    GcT_d = nc.dram_tensor("GcT", [FP, L], BF16, kind="ExternalInput")
    GsT_d = nc.dram_tensor("GsT", [FP, L], BF16, kind="ExternalInput")
    W_d = {}
    for nm in ("qr", "qi", "kr", "ki", "vr", "vi"):
        W_d[nm] = nc.dram_tensor(f"W{nm}", [E, 512], BF16, kind="ExternalInput")
        W_d["b" + nm] = nc.dram_tensor(f"b{nm}", [512, 1], F32, kind="ExternalInput")
    WoT_d = nc.dram_tensor("WoT", [512, E], BF16, kind="ExternalInput")
    out_d = nc.dram_tensor("out", [L, E], F32, kind="ExternalOutput")

    with tile.TileContext(nc) as tc:
        with tc.tile_pool(name="persist", bufs=1) as persist:
            # persistent outputs of projection phases
            Qcat = persist.tile([P, NH * FP], BF16)   # rows 0:64 real, 64:128 imag
            Kcat = persist.tile([P, NH * FP], BF16)
            Vc = [persist.tile([P, NH * 129], BF16, tag=f"Vc{i}", name=f"Vc{i}")
                  for i in range(len(MTI))]
            # constants (gpsimd DMA queue: keep the sync queue free for the
            # critical kv/fct loads)
            eps_t = persist.tile([P, 1], F32)
            nc.vector.memset(eps_t[:], EPS)
            gam8 = persist.tile([P, ET], F32)
            nc.gpsimd.dma_start(gam8[:], gamma_d.ap().rearrange("(c p) one -> p (c one)", p=P))
            bet8 = persist.tile([P, ET], F32)
            nc.gpsimd.dma_start(bet8[:], beta_d.ap().rearrange("(c p) one -> p (c one)", p=P))
            bias_t = {}
            for nm in ("qr", "qi", "kr", "ki"):
                bias_t[nm] = persist.tile([P, 4], F32, tag=f"b{nm}", name=f"b{nm}")
                nc.gpsimd.dma_start(bias_t[nm][:],
                                    W_d["b" + nm].ap().rearrange("(mt p) one -> p (mt one)", p=P))
            vbias = {}

            def load_w(names, wp):
                Wt = {}
                for nm in names:
                    Wt[nm] = wp.tile([P, ET * 512], BF16, tag=f"W{nm}", name=f"W{nm}")
                    nc.gpsimd.dma_start(
                        Wt[nm][:].rearrange("p (c e) -> p c e", c=ET),
                        W_d[nm].ap().rearrange("(c p) e -> p c e", p=P))
                return Wt

            def load_fslab(fsl, c):
                # split in lc-halves so the first DFT matmuls can start early
                f0 = c * CH
                fct = fsl.tile([P, LT * CH], BF16, tag="fct", name="fct")
                fst = fsl.tile([P, LT * CH], BF16, tag="fst", name="fst")
                hl = LT // 2
                for hf in range(2):
                    cs_ = slice(hf * hl * CH, (hf + 1) * hl * CH)
                    rs_ = slice(hf * hl * P, (hf + 1) * hl * P)
                    nc.sync.dma_start(
                        fct[:, cs_].rearrange("p (lc f) -> p lc f", lc=hl),
                        FcT_d.ap()[rs_, f0:f0 + CH].rearrange("(lc p) f -> p lc f", p=P))
                    nc.sync.dma_start(
                        fst[:, cs_].rearrange("p (lc f) -> p lc f", lc=hl),
                        FsT_d.ap()[rs_, f0:f0 + CH].rearrange("(lc p) f -> p lc f", p=P))
                return fct, fst

            # stage shuffle: [128(2hh x 64), 4mt*CH] staged -> cat[hh*64:(hh+1)*64, head cols]
            def stage_to_cat(cat, stage_r, stage_i, f0):
                # heads h = 2*mt + hh ; real rows 0:64 of cat, imag rows 64:128
                # (scalar DMA queue: off the critical sync queue)
                catv = cat.rearrange("p (h f) -> p h f", h=NH)
                for hh in range(2):
                    nc.scalar.dma_start(
                        catv[0:64, hh::2, f0:f0 + CH],
                        stage_r[hh * 64:(hh + 1) * 64, :].rearrange("p (mt f) -> p mt f", f=CH))
                    nc.scalar.dma_start(
                        catv[64:128, hh::2, f0:f0 + CH],
                        stage_i[hh * 64:(hh + 1) * 64, :].rearrange("p (mt f) -> p mt f", f=CH))

            def dft_chunk(src, fct, fst, evict):
                """lc-outer DFT accumulation in 2-eb groups; evict(eb, pr, pi)."""
                for g in range(ET // 2):
                    ebs = (2 * g, 2 * g + 1)
                    prs = {}
                    pis = {}
                    for eb in ebs:
                        prs[eb] = dps.tile([P, CH], F32, tag="pr", name="pr")
                        pis[eb] = dps.tile([P, CH], F32, tag="pi", name="pi")
                    for lc in range(LT):
                        for eb in ebs:
                            stat = src[:, lc * E + eb * P: lc * E + (eb + 1) * P]
                            nc.tensor.matmul(prs[eb][:], stat, fct[:, lc * CH:(lc + 1) * CH],
                                             start=(lc == 0), stop=(lc == LT - 1))
                            nc.tensor.matmul(pis[eb][:], stat, fst[:, lc * CH:(lc + 1) * CH],
                                             start=(lc == 0), stop=(lc == LT - 1))
                    for eb in ebs:
                        evict(eb, prs[eb], pis[eb])

            # =================== kv phase (+ LN of q overlapped) ===================
            with tc.tile_pool(name="qnsp", bufs=1) as qnsp, \
                 tc.tile_pool(name="qin", bufs=2) as qin, \
                 tc.tile_pool(name="lns", bufs=4) as lns, \
                 tc.tile_pool(name="fsl", bufs=1) as fsl, \
                 tc.tile_pool(name="xfp", bufs=1) as xfp, \
                 tc.tile_pool(name="stg", bufs=1) as stg, \
                 tc.tile_pool(name="dps", bufs=2, space="PSUM") as dps, \
                 tc.tile_pool(name="pps", bufs=4, space="PSUM") as pps:
                qns = qnsp.tile([P, LT * E], BF16)

                kvph_ctx = tc.tile_pool(name="kvph", bufs=1)
                kvph = kvph_ctx.__enter__()
                # kv slab in 4 quarter-DMAs so the first DFT matmuls start
                # as soon as the first quarter lands (subtile deps)
                kvs = kvph.tile([P, LT * E], BF16)
                for kq in range(4):
                    cs_ = slice(kq * 4 * E, (kq + 1) * 4 * E)
                    rs_ = slice(kq * 4 * P, (kq + 1) * 4 * P)
                    nc.sync.dma_start(kvs[:, cs_].rearrange("p (lc e) -> p lc e", lc=4),
                                      kv_d.ap()[rs_, :].rearrange("(lc p) e -> p lc e", p=P))

                cur_slab = load_fslab(fsl, 0)

                for nm in ("vr", "vi"):
                    vb_row = qin.tile([1, 512], F32, tag="qsl", name="vb_row")
                    nc.gpsimd.dma_start(vb_row[:], W_d["b" + nm].ap().rearrange("e one -> one e"))
                    vb = kvph.tile([P, 512], F32, tag=f"vb{nm}", name=f"vb{nm}")
                    nc.gpsimd.partition_broadcast(vb[:], vb_row[:])
                    vbias[nm] = vb

                Wkv = load_w(("kr", "ki", "vr", "vi"), kvph)

                # ---- LN instruction stream (DVE stats + ACT apply) ----
                for qq in range(8):
                    qsl = qin.tile([P, 2 * E], BF16, tag="qsl", name="qsl")
                    nc.sync.dma_start(
                        qsl[:].rearrange("p (lc e) -> p lc e", lc=2),
                        q_d.ap()[qq * 2 * P:(qq + 1) * 2 * P, :].rearrange(
                            "(lc p) e -> p lc e", p=P))
                    for lq in range(2):
                        lc = qq * 2 + lq
                        qt = qsl[:, lq * E:(lq + 1) * E]
                        st = lns.tile([P, 12], F32, tag="st", name="st")
                        nc.vector.bn_stats(st[:, 0:6], qt[:, 0:512])
                        nc.vector.bn_stats(st[:, 6:12], qt[:, 512:1024])
                        mv = lns.tile([P, 2], F32, tag="mv", name="mv")
                        nc.vector.bn_aggr(mv[:], st[:])
                        sd = lns.tile([P, 1], F32, tag="sd", name="sd")
                        nc.scalar.activation(sd[:], mv[:, 1:2], AF.Sqrt, bias=eps_t[:])
                        istd = lns.tile([P, 1], F32, tag="istd", name="istd")
                        nc.vector.reciprocal(istd[:], sd[:])
                        nmu = lns.tile([P, 1], F32, tag="nmu", name="nmu")
                        nc.vector.tensor_scalar_mul(nmu[:], mv[:, 0:1], -1.0)
                        nc.vector.tensor_mul(nmu[:], nmu[:], istd[:])
                        nc.scalar.activation(qns[:, lc * E:(lc + 1) * E], qt, AF.Identity,
                                             bias=nmu[:], scale=istd[:])

                # ---- kv DFT + K/V projections, per chunk ----
                for c in range(NCH):
                    fct, fst = cur_slab
                    xr = [None] * ET
                    xi = [None] * ET

                    def kv_evict(eb, pr, pi):
                        xr_ = xfp.tile([P, CH], BF16, tag=f"xr{eb}", name=f"xr{eb}")
                        xi_ = xfp.tile([P, CH], BF16, tag=f"xi{eb}", name=f"xi{eb}")
                        nc.scalar.activation(xr_[:], pr[:], AF.Copy)
                        nc.scalar.activation(xi_[:], pi[:], AF.Copy)
                        xr[eb] = xr_
                        xi[eb] = xi_

                    dft_chunk(kvs, fct, fst, kv_evict)
                    # prefetch next chunk's DFT matrices (overlaps K/V proj);
                    # after the last kv chunk, preload the q phase's chunk 0
                    cur_slab = load_fslab(fsl, c + 1 if c + 1 < NCH else 0)
                    # K projections
                    kr_st = stg.tile([P, 4 * CH], BF16, tag="kr_st", name="kr_st")
                    ki_st = stg.tile([P, 4 * CH], BF16, tag="ki_st", name="ki_st")
                    for mt in range(4):
                        pkr = pps.tile([P, CH], F32, tag="pp", name="pkr")
                        pki = pps.tile([P, CH], F32, tag="pp", name="pki")
                        for ec in range(ET):
                            nc.tensor.matmul(pkr[:], Wkv["kr"][:, ec * 512 + mt * P: ec * 512 + (mt + 1) * P],
                                             xr[ec][:], start=(ec == 0), stop=(ec == ET - 1))
                            nc.tensor.matmul(pki[:], Wkv["ki"][:, ec * 512 + mt * P: ec * 512 + (mt + 1) * P],
                                             xi[ec][:], start=(ec == 0), stop=(ec == ET - 1))
                        nc.scalar.activation(kr_st[:, mt * CH:(mt + 1) * CH], pkr[:],
                                             AF.Identity, bias=bias_t["kr"][:, mt:mt + 1])
                        nc.scalar.activation(ki_st[:, mt * CH:(mt + 1) * CH], pki[:],
                                             AF.Identity, bias=bias_t["ki"][:, mt:mt + 1])
                    stage_to_cat(Kcat, kr_st, ki_st, c * CH)
                    # V projections for the 3 m-tiles inside this chunk
                    for mi in (3 * c, 3 * c + 1, 3 * c + 2):
                        m0, msz = MTI[mi]
                        mr = m0 - c * CH
                        pvr = pps.tile([P, 512], F32, tag="pp", name="pvr")
                        pvi = pps.tile([P, 512], F32, tag="pp", name="pvi")
                        for ec in range(ET):
                            nc.tensor.matmul(pvr[0:msz, :], xr[ec][:, mr:mr + msz],
                                             Wkv["vr"][:, ec * 512:(ec + 1) * 512],
                                             start=(ec == 0), stop=(ec == ET - 1))
                            nc.tensor.matmul(pvi[0:msz, :], xi[ec][:, mr:mr + msz],
                                             Wkv["vi"][:, ec * 512:(ec + 1) * 512],
                                             start=(ec == 0), stop=(ec == ET - 1))
                        vco = Vc[mi][0:msz, :].rearrange("p (h c) -> p h c", h=NH)
                        nc.vector.tensor_add(
                            vco[:, :, 0:64],
                            pvr[0:msz, :].rearrange("p (h c) -> p h c", h=NH),
                            vbias["vr"][0:msz, :].rearrange("p (h c) -> p h c", h=NH))
                        nc.vector.tensor_add(
                            vco[:, :, 64:128],
                            pvi[0:msz, :].rearrange("p (h c) -> p h c", h=NH),
                            vbias["vi"][0:msz, :].rearrange("p (h c) -> p h c", h=NH))
                        nc.vector.memset(vco[:, :, 128:129], 1.0)

                # =================== q phase (kv-only tiles freed first) ===================
                kvph_ctx.__exit__(None, None, None)
                with tc.tile_pool(name="qph", bufs=1) as qph:
                    Wq = load_w(("qr", "qi"), qph)
                    for c in range(NCH):
                        fct, fst = cur_slab
                        xr = [None] * ET
                        xi = [None] * ET

                        def q_evict(eb, pr, pi, c=c):
                            xr_ = xfp.tile([P, CH], BF16, tag=f"xr{eb}", name=f"xr{eb}")
                            xi_ = xfp.tile([P, CH], BF16, tag=f"xi{eb}", name=f"xi{eb}")
                            # gamma folded in as per-partition (feature) scale
                            nc.scalar.activation(xr_[:], pr[:], AF.Identity,
                                                 scale=gam8[:, eb:eb + 1])
                            nc.scalar.activation(xi_[:], pi[:], AF.Identity,
                                                 scale=gam8[:, eb:eb + 1])
                            if c == 0:
                                # beta contributes only to DC (f=0) of the cos part
                                nc.vector.scalar_tensor_tensor(
                                    xr_[:, 0:1], bet8[:, eb:eb + 1], SQL,
                                    xr_[:, 0:1],
                                    op0=mybir.AluOpType.mult,
                                    op1=mybir.AluOpType.add)
                            xr[eb] = xr_
                            xi[eb] = xi_

                        dft_chunk(qns, fct, fst, q_evict)
                        if c + 1 < NCH:
                            cur_slab = load_fslab(fsl, c + 1)
                        qr_st = stg.tile([P, 4 * CH], BF16, tag="kr_st", name="qr_st")
                        qi_st = stg.tile([P, 4 * CH], BF16, tag="ki_st", name="qi_st")
                        for mt in range(4):
                            pqr = pps.tile([P, CH], F32, tag="pp", name="pqr")
                            pqi = pps.tile([P, CH], F32, tag="pp", name="pqi")
                            for ec in range(ET):
                                nc.tensor.matmul(pqr[:], Wq["qr"][:, ec * 512 + mt * P: ec * 512 + (mt + 1) * P],
                                                 xr[ec][:], start=(ec == 0), stop=(ec == ET - 1))
                                nc.tensor.matmul(pqi[:], Wq["qi"][:, ec * 512 + mt * P: ec * 512 + (mt + 1) * P],
                                                 xi[ec][:], start=(ec == 0), stop=(ec == ET - 1))
                            nc.scalar.activation(qr_st[:, mt * CH:(mt + 1) * CH], pqr[:],
                                                 AF.Identity, bias=bias_t["qr"][:, mt:mt + 1])
                            nc.scalar.activation(qi_st[:, mt * CH:(mt + 1) * CH], pqi[:],
                                                 AF.Identity, bias=bias_t["qi"][:, mt:mt + 1])
                        stage_to_cat(Qcat, qr_st, qi_st, c * CH)

            # =================== attention + iDFT + Wo ===================
            with tc.tile_pool(name="oacc", bufs=1) as oacc, \
                 tc.tile_pool(name="gsl", bufs=1) as gsl:
                our = []
                oui = []
                for ti in range(len(FTI)):
                    our.append(oacc.tile([P, 512], BF16, tag=f"our{ti}", name=f"our{ti}"))
                    oui.append(oacc.tile([P, 512], BF16, tag=f"oui{ti}", name=f"oui{ti}"))
                # prefetch iDFT matrices (half t-range only; mirror covers the rest)
                Gc = []
                Gs = []
                for ti, (m0, msz) in enumerate(FTI):
                    gc = gsl.tile([P, Lf], BF16, tag=f"gc{ti}", name=f"gc{ti}")
                    gs = gsl.tile([P, Lf], BF16, tag=f"gs{ti}", name=f"gs{ti}")
                    nc.sync.dma_start(gc[0:msz, :], GcT_d.ap()[m0:m0 + msz, 0:Lf])
                    nc.sync.dma_start(gs[0:msz, :], GsT_d.ap()[m0:m0 + msz, 0:Lf])
                    Gc.append(gc)
                    Gs.append(gs)

                with tc.tile_pool(name="expp", bufs=2) as expp, \
                     tc.tile_pool(name="sps", bufs=2, space="PSUM") as sps, \
                     tc.tile_pool(name="avps", bufs=2, space="PSUM") as avps, \
                     tc.tile_pool(name="nrm", bufs=4) as nrm:
                    for h in range(NH):
                        ets = []
                        for mi, (m0, msz) in enumerate(MTI):
                            ps = sps.tile([P, 1536], F32, tag="sc", name="sc")
                            for (s0, ssz) in ((0, 512), (512, 512), (1024, 2)):
                                nc.tensor.matmul(ps[0:msz, s0:s0 + ssz],
                                                 Kcat[:, h * FP + m0: h * FP + m0 + msz],
                                                 Qcat[:, h * FP + s0: h * FP + s0 + ssz],
                                                 start=True, stop=True)
                            et = expp.tile([P, FP], BF16, tag=f"e{mi}", name=f"e{mi}")
                            nc.scalar.activation(et[0:msz, :], ps[0:msz, 0:FP],
                                                 AF.Exp, scale=float(D ** -0.5))
                            ets.append(et)
                        for ti, (l0, lsz) in enumerate(FTI):
                            av = avps.tile([P, 129], F32, tag="av", name="av")
                            n = len(MTI)
                            for mi, (m0, msz) in enumerate(MTI):
                                nc.tensor.matmul(av[0:lsz, :], ets[mi][0:msz, l0:l0 + lsz],
                                                 Vc[mi][0:msz, h * 129:(h + 1) * 129],
                                                 start=(mi == 0), stop=(mi == n - 1))
                            rcp = nrm.tile([P, 1], F32, tag="rcp", name="rcp")
                            nc.vector.reciprocal(rcp[0:lsz, :], av[0:lsz, 128:129])
                            nc.vector.tensor_scalar_mul(our[ti][0:lsz, h * 64:(h + 1) * 64],
                                                        av[0:lsz, 0:64], rcp[0:lsz, :])
                            nc.vector.tensor_scalar_mul(oui[ti][0:lsz, h * 64:(h + 1) * 64],
                                                        av[0:lsz, 64:128], rcp[0:lsz, :])

                # ---------------- iDFT (half range + mirror) ----------------
                # x[t] = C[t] + S[t], x[2048-t] = C[t] - S[t]  (t = 1..1023)
                # where C = our . Gc, S = oui . Gs over f
                with tc.tile_pool(name="ottp", bufs=1) as ottp, \
                     tc.tile_pool(name="mirp", bufs=2) as mirp, \
                     tc.tile_pool(name="ops", bufs=2, space="PSUM") as ops:
                    OTT = [ottp.tile([P, L], BF16, tag=f"OTT{i}", name=f"OTT{i}")
                           for i in range(4)]
                    n = len(FTI)
                    for e4 in range(4):
                        for (t0, tn) in TCH:
                            pc = ops.tile([P, CH], F32, tag="pc", name="pc")
                            psn = ops.tile([P, CH], F32, tag="ps", name="psn")
                            for ti, (m0, msz) in enumerate(FTI):
                                nc.tensor.matmul(pc[:, 0:tn], our[ti][0:msz, e4 * P:(e4 + 1) * P],
                                                 Gc[ti][0:msz, t0:t0 + tn],
                                                 start=(ti == 0), stop=(ti == n - 1))
                                nc.tensor.matmul(psn[:, 0:tn], oui[ti][0:msz, e4 * P:(e4 + 1) * P],
                                                 Gs[ti][0:msz, t0:t0 + tn],
                                                 start=(ti == 0), stop=(ti == n - 1))
                            # DVE cannot read two PSUM operands: stage S in SBUF
                            s_sb = mirp.tile([P, CH], BF16, tag="ssb", name="s_sb")
                            nc.scalar.activation(s_sb[:, 0:tn], psn[:, 0:tn], AF.Copy)
                            nc.vector.tensor_add(OTT[e4][:, t0:t0 + tn], pc[:, 0:tn],
                                                 s_sb[:, 0:tn])
                            # mirror: skip t=0 (chunk 0) and t=1024 (chunk 2)
                            ml = t0 + tn - 1 if t0 + tn <= Lf - 1 else 1023
                            m_first = max(t0, 1)
                            stop_ = (m_first - t0) - 1 if m_first - t0 > 0 else None
                            # out cols [2048-ml .. 2048-m_first] <- src cols [ml .. m_first]
                            nc.vector.tensor_sub(
                                OTT[e4][:, L - ml:L - m_first + 1],
                                pc[:, (ml - t0):stop_:-1],
                                s_sb[:, (ml - t0):stop_:-1])

                    # ---------------- Wo ----------------
                    with tc.tile_pool(name="wop", bufs=1) as wop, \
                         tc.tile_pool(name="ost", bufs=3) as ost, \
                         tc.tile_pool(name="wops", bufs=4, space="PSUM") as wops:
                        WoT_t = wop.tile([P, 4 * E], BF16)
                        nc.sync.dma_start(WoT_t[:].rearrange("p (c e) -> p c e", c=4),
                                          WoT_d.ap().rearrange("(c p) e -> p c e", p=P))
                        for tb in range(LT):
                            ot_ = ost.tile([P, E], F32, tag="ot", name="ot")
                            for eo in range(2):
                                pso = wops.tile([P, 512], F32, tag="po2", name="pso")
                                for ec in range(4):
                                    nc.tensor.matmul(pso[:],
                                                     OTT[ec][:, tb * P:(tb + 1) * P],
                                                     WoT_t[:, ec * E + eo * 512: ec * E + (eo + 1) * 512],
                                                     start=(ec == 0), stop=(ec == 3))
                                if eo == 0:
                                    nc.scalar.activation(ot_[:, 0:512], pso[:], AF.Copy)
                                else:
                                    nc.vector.tensor_copy(ot_[:, 512:1024], pso[:])
                            nc.sync.dma_start(out_d.ap()[tb * P:(tb + 1) * P, :], ot_[:])

    nc.finalize()
    return nc


def kernel(**inputs):
    from concourse.bass_utils import run_bass_kernel_spmd

    if "nc" not in _CACHE:
        _CACHE["nc"] = _build()
        _CACHE["consts"] = _dft_consts()
    nc = _CACHE["nc"]
    FcT, FsT, GcT, GsT = _CACHE["consts"]

    rdt = ml_dtypes.bfloat16
    q = np.ascontiguousarray(inputs["query"], dtype=rdt)
    kv = np.ascontiguousarray(inputs["key_value"], dtype=rdt)
    in_maps = []
    for core in range(8):
        b = core // 2
        hg = core % 2
        cs = slice(hg * 512, (hg + 1) * 512)
        m = {
            "q": q[b],
            "kv": np.ascontiguousarray(kv[b]),
            "gamma": np.ascontiguousarray(inputs["gamma"], np.float32).reshape(E, 1),
            "beta": np.ascontiguousarray(inputs["beta"], np.float32).reshape(E, 1),
            "FcT": FcT.astype(rdt), "FsT": FsT.astype(rdt),
            "GcT": GcT.astype(rdt), "GsT": GsT.astype(rdt),
            "WoT": np.ascontiguousarray(inputs["Wo"][:, cs].T.astype(rdt)),
        }
        for nm in ("qr", "qi", "kr", "ki", "vr", "vi"):
            m[f"W{nm}"] = np.ascontiguousarray(inputs["W" + nm][cs, :].T.astype(rdt))
            m[f"b{nm}"] = np.ascontiguousarray(inputs["b" + nm][cs], np.float32).reshape(512, 1)
        in_maps.append(m)

    res = run_bass_kernel_spmd(nc, in_maps, core_ids=list(range(8)))
    _CACHE["last"] = res
    out = np.empty((B, L, E), np.float32)
    for b in range(B):
        out[b] = res.results[2 * b]["out"] + res.results[2 * b + 1]["out"]
    return out


# revision 38
# speedup vs baseline: 1.2262x; 1.0046x over previous
"""Trainium2 Bass kernel for nn_FreqCrossAttention.

Sharding: 8 cores = 4 batches x 2 head-groups (8 heads each).
Each core computes a partial output [2048, 1024] (its head-group's
contribution through W_o row-parallel); host sums the pair per batch.

v4: fully SBUF-resident pipeline (no DRAM round-trips for qn/Q/K/V),
slab DMA loads (2KB lines), LN overlapped under the kv-path DFT,
lc-outer DFT accumulation (matmuls chase the streaming input loads),
342-wide f-chunks (3 x 342 = 1026), chunk-aligned V/m-tiles, exp over
a 3-bank PSUM scores tile, iDFT computed for t=0..1024 only and
mirrored to the second half via cos/sin symmetry, all matmuls bf16.
"""
import math
import numpy as np
import ml_dtypes

B, L, E, H = 4, 2048, 1024, 16
D = E // H            # 64
Lf = L // 2 + 1       # 1025
FP = 1026             # padded frequency dim (= 3*342)
NH = 8                # heads per core
P = 128
CH = 342              # DFT/proj frequency chunk
NCH = 3
LT = 16               # L tiles of 128
ET = 8                # e-chunks of E
EPS = 1e-5
SQL = math.sqrt(L)
# m-tiles (keys), chunk-aligned: per chunk (0,128),(128,128),(256,86);
# the final tile is 85 rows (m=940..1024) so the pad key row 1025 is
# never part of the softmax.
MTI = []
for _c in range(NCH):
    last = 85 if _c == NCH - 1 else 86
    MTI += [(_c * CH, 128), (_c * CH + 128, 128), (_c * CH + 256, last)]
# l-tiles (queries/outputs) incl pad col 1025
FTI = [(i * P, P) for i in range(8)] + [(1024, 2)]
# iDFT half-range t-chunks (t = 0..1024); the rest mirrors
TCH = [(0, 342), (342, 342), (684, 341)]

_CACHE = {}


def _dft_consts():
    f = np.arange(FP)
    t = np.arange(L)
    ang = 2.0 * np.pi * np.outer(t, f) / L            # [L, FP]
    s = 1.0 / math.sqrt(L)
    FcT = (np.cos(ang) * s).astype(np.float32)        # rhs for rfft [L, FP]
    FsT = (-np.sin(ang) * s).astype(np.float32)
    FcT[:, Lf:] = 0.0
    FsT[:, Lf:] = 0.0
    cw = np.where((f == 0) | (f == L // 2), 1.0, 2.0)[:, None]
    GcT = (cw * np.cos(ang.T) * s).astype(np.float32)  # [FP, L]
    GsT = (-cw * np.sin(ang.T) * s).astype(np.float32)
    GcT[Lf:, :] = 0.0
    GsT[Lf:, :] = 0.0
    return FcT, FsT, GcT, GsT


def _build():
    import concourse.bass as bass
    import concourse.bacc as bacc
    import concourse.mybir as mybir
    import concourse.tile as tile

    F32 = mybir.dt.float32
    BF16 = mybir.dt.bfloat16
    AF = mybir.ActivationFunctionType

    nc = bacc.Bacc("TRN2", debug=False, num_devices=8)

    q_d = nc.dram_tensor("q", [L, E], BF16, kind="ExternalInput")
    kv_d = nc.dram_tensor("kv", [L, E], BF16, kind="ExternalInput")
    gamma_d = nc.dram_tensor("gamma", [E, 1], F32, kind="ExternalInput")
    beta_d = nc.dram_tensor("beta", [E, 1], F32, kind="ExternalInput")
    FcT_d = nc.dram_tensor("FcT", [L, FP], BF16, kind="ExternalInput")
    FsT_d = nc.dram_tensor("FsT", [L, FP], BF16, kind="ExternalInput")
    GcT_d = nc.dram_tensor("GcT", [FP, L], BF16, kind="ExternalInput")
    GsT_d = nc.dram_tensor("GsT", [FP, L], BF16, kind="ExternalInput")
    W_d = {}
    for nm in ("qr", "qi", "kr", "ki", "vr", "vi"):
        W_d[nm] = nc.dram_tensor(f"W{nm}", [E, 512], BF16, kind="ExternalInput")
        W_d["b" + nm] = nc.dram_tensor(f"b{nm}", [512, 1], F32, kind="ExternalInput")
    WoT_d = nc.dram_tensor("WoT", [512, E], BF16, kind="ExternalInput")
    out_d = nc.dram_tensor("out", [L, E], F32, kind="ExternalOutput")

    with tile.TileContext(nc) as tc:
        with tc.tile_pool(name="persist", bufs=1) as persist:
            # persistent outputs of projection phases
            Qcat = persist.tile([P, NH * FP], BF16)   # rows 0:64 real, 64:128 imag
            Kcat = persist.tile([P, NH * FP], BF16)
            Vc = [persist.tile([P, NH * 129], BF16, tag=f"Vc{i}", name=f"Vc{i}")
                  for i in range(len(MTI))]
            # constants (gpsimd DMA queue: keep the sync queue free for the
            # critical kv/fct loads)
            eps_t = persist.tile([P, 1], F32)
            nc.vector.memset(eps_t[:], EPS)
            gam8 = persist.tile([P, ET], F32)
            nc.gpsimd.dma_start(gam8[:], gamma_d.ap().rearrange("(c p) one -> p (c one)", p=P))
            bet8 = persist.tile([P, ET], F32)
            nc.gpsimd.dma_start(bet8[:], beta_d.ap().rearrange("(c p) one -> p (c one)", p=P))
            bias_t = {}
            for nm in ("qr", "qi", "kr", "ki"):
                bias_t[nm] = persist.tile([P, 4], F32, tag=f"b{nm}", name=f"b{nm}")
                nc.gpsimd.dma_start(bias_t[nm][:],
                                    W_d["b" + nm].ap().rearrange("(mt p) one -> p (mt one)", p=P))
            vbias = {}

            def load_w(names, wp):
                Wt = {}
                for nm in names:
                    Wt[nm] = wp.tile([P, ET * 512], BF16, tag=f"W{nm}", name=f"W{nm}")
                    nc.gpsimd.dma_start(
                        Wt[nm][:].rearrange("p (c e) -> p c e", c=ET),
                        W_d[nm].ap().rearrange("(c p) e -> p c e", p=P))
                return Wt

            def load_fslab(fsl, c):
                # split in lc-halves so the first DFT matmuls can start early
                f0 = c * CH
                fct = fsl.tile([P, LT * CH], BF16, tag="fct", name="fct")
                fst = fsl.tile([P, LT * CH], BF16, tag="fst", name="fst")
                hl = LT // 2
                for hf in range(2):
                    cs_ = slice(hf * hl * CH, (hf + 1) * hl * CH)
                    rs_ = slice(hf * hl * P, (hf + 1) * hl * P)
                    nc.sync.dma_start(
                        fct[:, cs_].rearrange("p (lc f) -> p lc f", lc=hl),
                        FcT_d.ap()[rs_, f0:f0 + CH].rearrange("(lc p) f -> p lc f", p=P))
                    nc.sync.dma_start(
                        fst[:, cs_].rearrange("p (lc f) -> p lc f", lc=hl),
                        FsT_d.ap()[rs_, f0:f0 + CH].rearrange("(lc p) f -> p lc f", p=P))
                return fct, fst

            # stage shuffle: [128(2hh x 64), 4mt*CH] staged -> cat[hh*64:(hh+1)*64, head cols]
            def stage_to_cat(cat, stage_r, stage_i, f0):
                # heads h = 2*mt + hh ; real rows 0:64 of cat, imag rows 64:128
                # (scalar DMA queue: off the critical sync queue)
                catv = cat.rearrange("p (h f) -> p h f", h=NH)
                for hh in range(2):
                    nc.scalar.dma_start(
                        catv[0:64, hh::2, f0:f0 + CH],
                        stage_r[hh * 64:(hh + 1) * 64, :].rearrange("p (mt f) -> p mt f", f=CH))
                    nc.scalar.dma_start(
                        catv[64:128, hh::2, f0:f0 + CH],
                        stage_i[hh * 64:(hh + 1) * 64, :].rearrange("p (mt f) -> p mt f", f=CH))

            def dft_chunk(src, fct, fst, evict):
                """lc-outer DFT accumulation in 2-eb groups; evict(eb, pr, pi)."""
                for g in range(ET // 2):
                    ebs = (2 * g, 2 * g + 1)
                    prs = {}
                    pis = {}
                    for eb in ebs:
                        prs[eb] = dps.tile([P, CH], F32, tag="pr", name="pr")
                        pis[eb] = dps.tile([P, CH], F32, tag="pi", name="pi")
                    for lc in range(LT):
                        for eb in ebs:
                            stat = src[:, lc * E + eb * P: lc * E + (eb + 1) * P]
                            nc.tensor.matmul(prs[eb][:], stat, fct[:, lc * CH:(lc + 1) * CH],
                                             start=(lc == 0), stop=(lc == LT - 1))
                            nc.tensor.matmul(pis[eb][:], stat, fst[:, lc * CH:(lc + 1) * CH],
                                             start=(lc == 0), stop=(lc == LT - 1))
                    for eb in ebs:
                        evict(eb, prs[eb], pis[eb])

            # =================== kv phase (+ LN of q overlapped) ===================
            with tc.tile_pool(name="qnsp", bufs=1) as qnsp, \
                 tc.tile_pool(name="qin", bufs=2) as qin, \
                 tc.tile_pool(name="lns", bufs=4) as lns, \
                 tc.tile_pool(name="fsl", bufs=1) as fsl, \
                 tc.tile_pool(name="xfp", bufs=1) as xfp, \
                 tc.tile_pool(name="stg", bufs=1) as stg, \
                 tc.tile_pool(name="dps", bufs=2, space="PSUM") as dps, \
                 tc.tile_pool(name="pps", bufs=4, space="PSUM") as pps:
                qns = qnsp.tile([P, LT * E], BF16)

                kvph_ctx = tc.tile_pool(name="kvph", bufs=1)
                kvph = kvph_ctx.__enter__()
                # interleave kv-slab quarters with DFT-matrix slab halves so
                # the first DFT matmuls can start ~10us in (subtile deps)
                kvs = kvph.tile([P, LT * E], BF16)
                fct0 = fsl.tile([P, LT * CH], BF16, tag="fct", name="fct")
                fst0 = fsl.tile([P, LT * CH], BF16, tag="fst", name="fst")
                hl = LT // 2
                for kq in range(4):
                    cs_ = slice(kq * 4 * E, (kq + 1) * 4 * E)
                    rs_ = slice(kq * 4 * P, (kq + 1) * 4 * P)
                    nc.sync.dma_start(kvs[:, cs_].rearrange("p (lc e) -> p lc e", lc=4),
                                      kv_d.ap()[rs_, :].rearrange("(lc p) e -> p lc e", p=P))
                    if kq < 2:
                        fcs_ = slice(kq * hl * CH, (kq + 1) * hl * CH)
                        frs_ = slice(kq * hl * P, (kq + 1) * hl * P)
                        nc.sync.dma_start(
                            fct0[:, fcs_].rearrange("p (lc f) -> p lc f", lc=hl),
                            FcT_d.ap()[frs_, 0:CH].rearrange("(lc p) f -> p lc f", p=P))
                        nc.sync.dma_start(
                            fst0[:, fcs_].rearrange("p (lc f) -> p lc f", lc=hl),
                            FsT_d.ap()[frs_, 0:CH].rearrange("(lc p) f -> p lc f", p=P))
                cur_slab = (fct0, fst0)

                for nm in ("vr", "vi"):
                    vb_row = qin.tile([1, 512], F32, tag="qsl", name="vb_row")
                    nc.gpsimd.dma_start(vb_row[:], W_d["b" + nm].ap().rearrange("e one -> one e"))
                    vb = kvph.tile([P, 512], F32, tag=f"vb{nm}", name=f"vb{nm}")
                    nc.gpsimd.partition_broadcast(vb[:], vb_row[:])
                    vbias[nm] = vb

                Wkv = load_w(("kr", "ki", "vr", "vi"), kvph)

                # ---- LN instruction stream (DVE stats + ACT apply) ----
                for qq in range(8):
                    qsl = qin.tile([P, 2 * E], BF16, tag="qsl", name="qsl")
                    nc.sync.dma_start(
                        qsl[:].rearrange("p (lc e) -> p lc e", lc=2),
                        q_d.ap()[qq * 2 * P:(qq + 1) * 2 * P, :].rearrange(
                            "(lc p) e -> p lc e", p=P))
                    for lq in range(2):
                        lc = qq * 2 + lq
                        qt = qsl[:, lq * E:(lq + 1) * E]
                        st = lns.tile([P, 12], F32, tag="st", name="st")
                        nc.vector.bn_stats(st[:, 0:6], qt[:, 0:512])
                        nc.vector.bn_stats(st[:, 6:12], qt[:, 512:1024])
                        mv = lns.tile([P, 2], F32, tag="mv", name="mv")
                        nc.vector.bn_aggr(mv[:], st[:])
                        sd = lns.tile([P, 1], F32, tag="sd", name="sd")
                        nc.scalar.activation(sd[:], mv[:, 1:2], AF.Sqrt, bias=eps_t[:])
                        istd = lns.tile([P, 1], F32, tag="istd", name="istd")
                        nc.vector.reciprocal(istd[:], sd[:])
                        nmu = lns.tile([P, 1], F32, tag="nmu", name="nmu")
                        nc.vector.tensor_scalar_mul(nmu[:], mv[:, 0:1], -1.0)
                        nc.vector.tensor_mul(nmu[:], nmu[:], istd[:])
                        nc.scalar.activation(qns[:, lc * E:(lc + 1) * E], qt, AF.Identity,
                                             bias=nmu[:], scale=istd[:])

                # ---- kv DFT + K/V projections, per chunk ----
                for c in range(NCH):
                    fct, fst = cur_slab
                    xr = [None] * ET
                    xi = [None] * ET

                    def kv_evict(eb, pr, pi):
                        xr_ = xfp.tile([P, CH], BF16, tag=f"xr{eb}", name=f"xr{eb}")
                        xi_ = xfp.tile([P, CH], BF16, tag=f"xi{eb}", name=f"xi{eb}")
                        nc.scalar.activation(xr_[:], pr[:], AF.Copy)
                        nc.scalar.activation(xi_[:], pi[:], AF.Copy)
                        xr[eb] = xr_
                        xi[eb] = xi_

                    dft_chunk(kvs, fct, fst, kv_evict)
                    # prefetch next chunk's DFT matrices (overlaps K/V proj);
                    # after the last kv chunk, preload the q phase's chunk 0
                    cur_slab = load_fslab(fsl, c + 1 if c + 1 < NCH else 0)
                    # K projections
                    kr_st = stg.tile([P, 4 * CH], BF16, tag="kr_st", name="kr_st")
                    ki_st = stg.tile([P, 4 * CH], BF16, tag="ki_st", name="ki_st")
                    for mt in range(4):
                        pkr = pps.tile([P, CH], F32, tag="pp", name="pkr")
                        pki = pps.tile([P, CH], F32, tag="pp", name="pki")
                        for ec in range(ET):
                            nc.tensor.matmul(pkr[:], Wkv["kr"][:, ec * 512 + mt * P: ec * 512 + (mt + 1) * P],
                                             xr[ec][:], start=(ec == 0), stop=(ec == ET - 1))
                            nc.tensor.matmul(pki[:], Wkv["ki"][:, ec * 512 + mt * P: ec * 512 + (mt + 1) * P],
                                             xi[ec][:], start=(ec == 0), stop=(ec == ET - 1))
                        nc.scalar.activation(kr_st[:, mt * CH:(mt + 1) * CH], pkr[:],
                                             AF.Identity, bias=bias_t["kr"][:, mt:mt + 1])
                        nc.scalar.activation(ki_st[:, mt * CH:(mt + 1) * CH], pki[:],
                                             AF.Identity, bias=bias_t["ki"][:, mt:mt + 1])
                    stage_to_cat(Kcat, kr_st, ki_st, c * CH)
                    # V projections for the 3 m-tiles inside this chunk
                    for mi in (3 * c, 3 * c + 1, 3 * c + 2):
                        m0, msz = MTI[mi]
                        mr = m0 - c * CH
                        pvr = pps.tile([P, 512], F32, tag="pp", name="pvr")
                        pvi = pps.tile([P, 512], F32, tag="pp", name="pvi")
                        for ec in range(ET):
                            nc.tensor.matmul(pvr[0:msz, :], xr[ec][:, mr:mr + msz],
                                             Wkv["vr"][:, ec * 512:(ec + 1) * 512],
                                             start=(ec == 0), stop=(ec == ET - 1))
                            nc.tensor.matmul(pvi[0:msz, :], xi[ec][:, mr:mr + msz],
                                             Wkv["vi"][:, ec * 512:(ec + 1) * 512],
                                             start=(ec == 0), stop=(ec == ET - 1))
                        vco = Vc[mi][0:msz, :].rearrange("p (h c) -> p h c", h=NH)
                        nc.vector.tensor_add(
                            vco[:, :, 0:64],
                            pvr[0:msz, :].rearrange("p (h c) -> p h c", h=NH),
                            vbias["vr"][0:msz, :].rearrange("p (h c) -> p h c", h=NH))
                        nc.vector.tensor_add(
                            vco[:, :, 64:128],
                            pvi[0:msz, :].rearrange("p (h c) -> p h c", h=NH),
                            vbias["vi"][0:msz, :].rearrange("p (h c) -> p h c", h=NH))
                        nc.vector.memset(vco[:, :, 128:129], 1.0)

                # =================== q phase (kv-only tiles freed first) ===================
                kvph_ctx.__exit__(None, None, None)
                with tc.tile_pool(name="qph", bufs=1) as qph:
                    Wq = load_w(("qr", "qi"), qph)
                    for c in range(NCH):
                        fct, fst = cur_slab
                        xr = [None] * ET
                        xi = [None] * ET

                        def q_evict(eb, pr, pi, c=c):
                            xr_ = xfp.tile([P, CH], BF16, tag=f"xr{eb}", name=f"xr{eb}")
                            xi_ = xfp.tile([P, CH], BF16, tag=f"xi{eb}", name=f"xi{eb}")
                            # gamma folded in as per-partition (feature) scale
                            nc.scalar.activation(xr_[:], pr[:], AF.Identity,
                                                 scale=gam8[:, eb:eb + 1])
                            nc.scalar.activation(xi_[:], pi[:], AF.Identity,
                                                 scale=gam8[:, eb:eb + 1])
                            if c == 0:
                                # beta contributes only to DC (f=0) of the cos part
                                nc.vector.scalar_tensor_tensor(
                                    xr_[:, 0:1], bet8[:, eb:eb + 1], SQL,
                                    xr_[:, 0:1],
                                    op0=mybir.AluOpType.mult,
                                    op1=mybir.AluOpType.add)
                            xr[eb] = xr_
                            xi[eb] = xi_

                        dft_chunk(qns, fct, fst, q_evict)
                        if c + 1 < NCH:
                            cur_slab = load_fslab(fsl, c + 1)
                        qr_st = stg.tile([P, 4 * CH], BF16, tag="kr_st", name="qr_st")
                        qi_st = stg.tile([P, 4 * CH], BF16, tag="ki_st", name="qi_st")
                        for mt in range(4):
                            pqr = pps.tile([P, CH], F32, tag="pp", name="pqr")
                            pqi = pps.tile([P, CH], F32, tag="pp", name="pqi")
                            for ec in range(ET):
                                nc.tensor.matmul(pqr[:], Wq["qr"][:, ec * 512 + mt * P: ec * 512 + (mt + 1) * P],
                                                 xr[ec][:], start=(ec == 0), stop=(ec == ET - 1))
                                nc.tensor.matmul(pqi[:], Wq["qi"][:, ec * 512 + mt * P: ec * 512 + (mt + 1) * P],
                                                 xi[ec][:], start=(ec == 0), stop=(ec == ET - 1))
                            nc.scalar.activation(qr_st[:, mt * CH:(mt + 1) * CH], pqr[:],
                                                 AF.Identity, bias=bias_t["qr"][:, mt:mt + 1])
                            nc.scalar.activation(qi_st[:, mt * CH:(mt + 1) * CH], pqi[:],
                                                 AF.Identity, bias=bias_t["qi"][:, mt:mt + 1])
                        stage_to_cat(Qcat, qr_st, qi_st, c * CH)

            # =================== attention + iDFT + Wo ===================
            with tc.tile_pool(name="oacc", bufs=1) as oacc, \
                 tc.tile_pool(name="gsl", bufs=1) as gsl:
                our = []
                oui = []
                for ti in range(len(FTI)):
                    our.append(oacc.tile([P, 512], BF16, tag=f"our{ti}", name=f"our{ti}"))
                    oui.append(oacc.tile([P, 512], BF16, tag=f"oui{ti}", name=f"oui{ti}"))
                # prefetch iDFT matrices (half t-range only; mirror covers the rest)
                Gc = []
                Gs = []
                for ti, (m0, msz) in enumerate(FTI):
                    gc = gsl.tile([P, Lf], BF16, tag=f"gc{ti}", name=f"gc{ti}")
                    gs = gsl.tile([P, Lf], BF16, tag=f"gs{ti}", name=f"gs{ti}")
                    nc.sync.dma_start(gc[0:msz, :], GcT_d.ap()[m0:m0 + msz, 0:Lf])
                    nc.sync.dma_start(gs[0:msz, :], GsT_d.ap()[m0:m0 + msz, 0:Lf])
                    Gc.append(gc)
                    Gs.append(gs)

                with tc.tile_pool(name="expp", bufs=2) as expp, \
                     tc.tile_pool(name="sps", bufs=2, space="PSUM") as sps, \
                     tc.tile_pool(name="avps", bufs=2, space="PSUM") as avps, \
                     tc.tile_pool(name="nrm", bufs=4) as nrm:
                    for h in range(NH):
                        ets = []
                        for mi, (m0, msz) in enumerate(MTI):
                            ps = sps.tile([P, 1536], F32, tag="sc", name="sc")
                            for (s0, ssz) in ((0, 512), (512, 512), (1024, 2)):
                                nc.tensor.matmul(ps[0:msz, s0:s0 + ssz],
                                                 Kcat[:, h * FP + m0: h * FP + m0 + msz],
                                                 Qcat[:, h * FP + s0: h * FP + s0 + ssz],
                                                 start=True, stop=True)
                            et = expp.tile([P, FP], BF16, tag=f"e{mi}", name=f"e{mi}")
                            nc.scalar.activation(et[0:msz, :], ps[0:msz, 0:FP],
                                                 AF.Exp, scale=float(D ** -0.5))
                            ets.append(et)
                        for ti, (l0, lsz) in enumerate(FTI):
                            av = avps.tile([P, 129], F32, tag="av", name="av")
                            n = len(MTI)
                            for mi, (m0, msz) in enumerate(MTI):
                                nc.tensor.matmul(av[0:lsz, :], ets[mi][0:msz, l0:l0 + lsz],
                                                 Vc[mi][0:msz, h * 129:(h + 1) * 129],
                                                 start=(mi == 0), stop=(mi == n - 1))
                            rcp = nrm.tile([P, 1], F32, tag="rcp", name="rcp")
                            nc.vector.reciprocal(rcp[0:lsz, :], av[0:lsz, 128:129])
                            nc.vector.tensor_scalar_mul(our[ti][0:lsz, h * 64:(h + 1) * 64],
                                                        av[0:lsz, 0:64], rcp[0:lsz, :])
                            nc.vector.tensor_scalar_mul(oui[ti][0:lsz, h * 64:(h + 1) * 64],
                                                        av[0:lsz, 64:128], rcp[0:lsz, :])

                # ---------------- iDFT (half range + mirror) ----------------
                # x[t] = C[t] + S[t], x[2048-t] = C[t] - S[t]  (t = 1..1023)
                # where C = our . Gc, S = oui . Gs over f
                with tc.tile_pool(name="ottp", bufs=1) as ottp, \
                     tc.tile_pool(name="mirp", bufs=2) as mirp, \
                     tc.tile_pool(name="ops", bufs=2, space="PSUM") as ops:
                    OTT = [ottp.tile([P, L], BF16, tag=f"OTT{i}", name=f"OTT{i}")
                           for i in range(4)]
                    n = len(FTI)
                    for e4 in range(4):
                        for (t0, tn) in TCH:
                            pc = ops.tile([P, CH], F32, tag="pc", name="pc")
                            psn = ops.tile([P, CH], F32, tag="ps", name="psn")
                            for ti, (m0, msz) in enumerate(FTI):
                                nc.tensor.matmul(pc[:, 0:tn], our[ti][0:msz, e4 * P:(e4 + 1) * P],
                                                 Gc[ti][0:msz, t0:t0 + tn],
                                                 start=(ti == 0), stop=(ti == n - 1))
                                nc.tensor.matmul(psn[:, 0:tn], oui[ti][0:msz, e4 * P:(e4 + 1) * P],
                                                 Gs[ti][0:msz, t0:t0 + tn],
                                                 start=(ti == 0), stop=(ti == n - 1))
                            # DVE cannot read two PSUM operands: stage S in SBUF
                            s_sb = mirp.tile([P, CH], BF16, tag="ssb", name="s_sb")
                            nc.scalar.activation(s_sb[:, 0:tn], psn[:, 0:tn], AF.Copy)
                            nc.vector.tensor_add(OTT[e4][:, t0:t0 + tn], pc[:, 0:tn],
                                                 s_sb[:, 0:tn])
                            # mirror: skip t=0 (chunk 0) and t=1024 (chunk 2)
                            ml = t0 + tn - 1 if t0 + tn <= Lf - 1 else 1023
                            m_first = max(t0, 1)
                            stop_ = (m_first - t0) - 1 if m_first - t0 > 0 else None
                            # out cols [2048-ml .. 2048-m_first] <- src cols [ml .. m_first]
                            nc.vector.tensor_sub(
                                OTT[e4][:, L - ml:L - m_first + 1],
                                pc[:, (ml - t0):stop_:-1],
                                s_sb[:, (ml - t0):stop_:-1])

                    # ---------------- Wo ----------------
                    with tc.tile_pool(name="wop", bufs=1) as wop, \
                         tc.tile_pool(name="ost", bufs=3) as ost, \
                         tc.tile_pool(name="wops", bufs=4, space="PSUM") as wops:
                        WoT_t = wop.tile([P, 4 * E], BF16)
                        nc.sync.dma_start(WoT_t[:].rearrange("p (c e) -> p c e", c=4),
                                          WoT_d.ap().rearrange("(c p) e -> p c e", p=P))
                        for tb in range(LT):
                            ot_ = ost.tile([P, E], F32, tag="ot", name="ot")
                            for eo in range(2):
                                pso = wops.tile([P, 512], F32, tag="po2", name="pso")
                                for ec in range(4):
                                    nc.tensor.matmul(pso[:],
                                                     OTT[ec][:, tb * P:(tb + 1) * P],
                                                     WoT_t[:, ec * E + eo * 512: ec * E + (eo + 1) * 512],
                                                     start=(ec == 0), stop=(ec == 3))
                                if eo == 0:
                                    nc.scalar.activation(ot_[:, 0:512], pso[:], AF.Copy)
                                else:
                                    nc.vector.tensor_copy(ot_[:, 512:1024], pso[:])
                            nc.sync.dma_start(out_d.ap()[tb * P:(tb + 1) * P, :], ot_[:])

    nc.finalize()
    return nc


def kernel(**inputs):
    from concourse.bass_utils import run_bass_kernel_spmd

    if "nc" not in _CACHE:
        _CACHE["nc"] = _build()
        _CACHE["consts"] = _dft_consts()
    nc = _CACHE["nc"]
    FcT, FsT, GcT, GsT = _CACHE["consts"]

    rdt = ml_dtypes.bfloat16
    q = np.ascontiguousarray(inputs["query"], dtype=rdt)
    kv = np.ascontiguousarray(inputs["key_value"], dtype=rdt)
    in_maps = []
    for core in range(8):
        b = core // 2
        hg = core % 2
        cs = slice(hg * 512, (hg + 1) * 512)
        m = {
            "q": q[b],
            "kv": np.ascontiguousarray(kv[b]),
            "gamma": np.ascontiguousarray(inputs["gamma"], np.float32).reshape(E, 1),
            "beta": np.ascontiguousarray(inputs["beta"], np.float32).reshape(E, 1),
            "FcT": FcT.astype(rdt), "FsT": FsT.astype(rdt),
            "GcT": GcT.astype(rdt), "GsT": GsT.astype(rdt),
            "WoT": np.ascontiguousarray(inputs["Wo"][:, cs].T.astype(rdt)),
        }
        for nm in ("qr", "qi", "kr", "ki", "vr", "vi"):
            m[f"W{nm}"] = np.ascontiguousarray(inputs["W" + nm][cs, :].T.astype(rdt))
            m[f"b{nm}"] = np.ascontiguousarray(inputs["b" + nm][cs], np.float32).reshape(512, 1)
        in_maps.append(m)

    res = run_bass_kernel_spmd(nc, in_maps, core_ids=list(range(8)))
    _CACHE["last"] = res
    out = np.empty((B, L, E), np.float32)
    for b in range(B):
        out[b] = res.results[2 * b]["out"] + res.results[2 * b + 1]["out"]
    return out
